# revision 2
# baseline (speedup 1.0000x reference)
"""Causal self-attention (B=4, T=2048, D=2048, H=16, HD=128) on 8 Trainium2
NeuronCores.

Sharding: Megatron-style tensor parallel over heads for QKV projection +
attention (2 heads per core), then on-device AllToAlls reshard from
head-parallel to token-parallel (core j owns tokens of batch j//2, half j%2)
for the output projection.  Host only slices/transposes/quantizes weights,
replicates activations, and concatenates the 8 output shards.

The two K=2048 GEMMs (QKV projection, output projection) run as fp8e4
DoubleRow matmuls (2x PE rate) with a 3-term hi/lo expansion:
  W ~ Wh + Wl,  X ~ Xh + Xl   (same power-of-2 scale for hi and lo)
  W.T X ~ Wh.T Xh + Wh.T Xl + Wl.T Xh     (lo.T lo dropped, ~0.06% error)
Each DoubleRow instruction packs two fp8 products over a pair of adjacent
128-deep contraction blocks, so the 3-term expansion costs 24 DoubleRow
instructions where bf16 needed 16 plain ones: 1.33x fewer PE cycles at
BETTER-than-bf16 accuracy (the hi/lo pair carries ~7 mantissa bits).
Attention itself (scores, AV) stays bf16: its K=128 contractions can't
pair the odd 3rd term, and fp8-quantizing P would bottleneck DVE/ACT.

Device layouts (PSUM always fp32):
  xT    [D, 2, B*T] fp8 hi/lo, x transposed (contraction dim on partitions)
  qT/kT [128, T] bf16 per (local head, batch); d-order permuted so the RoPE
                 rotate-half partner sits 16 partitions away.
  V     [T, 128] bf16 natural d order
  S^T   [tk, tq] scores transposed; no max-subtraction needed.
  oT    fp8 hi/lo pair (scale 32), so the AllToAll moves the same bytes as
        bf16 but lands ready for the fp8 out-projection.

The softmax denominator is accumulated on the DVE and reduced across
partitions by a single 128-column-stationary (1/32)-matmul per 512-query
chunk (the 1/32 folds the fp8 scale of oT into the reciprocal for free).

The attention loop runs tq-half 0 then half 1, with one AllToAll per
(half, head) issued as soon as that head's chunks finish - all four
collectives overlap the remaining attention / output projection.
"""

import sys

for _p in ("/opt/trn_rl_repo", "/root/.axon_site/_ro/trn_rl_repo"):
    if _p not in sys.path:
        sys.path.insert(0, _p)

import numpy as np
import ml_dtypes

BF16 = ml_dtypes.bfloat16
E4M3 = ml_dtypes.float8_e4m3  # TRN float8e4 (max normal 240)

B = 4
D = 2048
H = 16
HD = 128
NCORES = 8
HL = 2           # heads per core
CB = D // 128    # contraction blocks
NJP = CB // 2    # contraction block pairs (DoubleRow)
TCH = 512        # token chunk (matmul moving free dim)
PAN = 1024       # x DMA panel (2 chunks)

SX = 32.0        # fp8 scale for x (|x|max ~5.7 -> 182 < 240)
SW = 1024.0      # fp8 scale for w_qkv / w_out (|w|max ~0.12 -> 125)
SA = 32.0        # fp8 scale for attention outputs (|o|max <= |v|max ~5.5)
QKV_DESC = 1.0 / (SX * SW)   # fold-out for q/k/v psum
OUT_DESC = 1.0 / (SA * SW)   # fold-out for out-proj psum


def _perm128():
    """Partition order for q/k head dims: quadrant g holds dims
    [16g,16g+16) (lo) then [64+16g, 64+16g+16) (hi), so the rotate-half
    partner of partition p is p+-16 (same 32-partition quadrant)."""
    perm = np.zeros(128, np.int64)
    for p in range(128):
        g, i = divmod(p, 32)
        perm[p] = g * 16 + i if i < 16 else 64 + g * 16 + (i - 16)
    return perm


_PERM = _perm128()
_SHUF = [(i + 16) % 32 for i in range(32)]  # out[i] = in[(i+16)%32]
_SIGN = np.where(np.arange(128) % 32 < 16, -1.0, 1.0).astype(np.float32)


def build_nc(T=2048):
    import concourse.bacc as bacc
    import concourse.tile as tile
    import concourse.mybir as mybir

    f32 = mybir.dt.float32
    bf16 = mybir.dt.bfloat16
    fp8 = mybir.dt.float8e4
    DR = mybir.MatmulPerfMode.DoubleRow
    TOK = B * T
    THALF = T // 2
    TQ = THALF // 2           # tokens per (core, a2a part)
    NPAN = TOK // PAN         # x panels total
    CHB = T // TCH            # token chunks per batch
    TB = T // 128             # 128-token blocks per batch
    SCALE = float(HD) ** -0.5
    Exp = mybir.ActivationFunctionType.Exp

    assert TQ == TCH, "A2A split layout assumes T == 2048"
    nc = bacc.Bacc("TRN2", target_bir_lowering=False, debug=False,
                   num_devices=NCORES)

    # x rows: [pan, hi/lo, t] per d-row;  wqk rows: [hi/lo, f]
    xT_d = nc.dram_tensor("xT2", [D, 2 * TOK], fp8, kind="ExternalInput")
    wqkT_d = nc.dram_tensor("wqkT2", [D, 2 * 6 * HD], fp8,
                            kind="ExternalInput")
    # wout rows: [oc, hi/lo, o]
    woutT_d = nc.dram_tensor("woutT2", [D, 2 * D], fp8, kind="ExternalInput")
    cosT_d = nc.dram_tensor("cosT", [HD, T], bf16, kind="ExternalInput")
    sinS_d = nc.dram_tensor("sinS", [HD, T], bf16, kind="ExternalInput")
    out_d = nc.dram_tensor("out", [THALF, D], f32, kind="ExternalOutput")

    xT_v = xT_d.ap().rearrange("(cb p) (pan two t) -> p cb pan two t",
                               p=128, two=2, t=PAN)
    wqkT_v = wqkT_d.ap().rearrange("(cb p) (two f) -> p cb two f",
                                   p=128, two=2)
    woutT_v = woutT_d.ap().rearrange("(cb p) (oc two o) -> p cb oc two o",
                                     p=128, two=2, o=TCH)

    with tile.TileContext(nc) as tc:
        with (
            tc.tile_pool(name="const", bufs=1) as constp,
            tc.tile_pool(name="dram", bufs=1, space="DRAM") as dramp,
        ):
            cos_sb = constp.tile([128, T], bf16, name="cos_sb")
            sin_sb = constp.tile([128, T], bf16, name="sin_sb")
            mask_sb = constp.tile([128, 4, TCH], bf16, name="mask_sb")
            ones_sb = constp.tile([128, 128], bf16, name="ones_sb")
            nc.gpsimd.memset(mask_sb[:], 1.0)
            for jd in range(4):
                # keep 1.0 where  tq_rel - tk_rel - 128*jd >= 0  else 0
                nc.gpsimd.affine_select(
                    out=mask_sb[:, jd, :], in_=mask_sb[:, jd, :],
                    compare_op=mybir.AluOpType.is_ge, fill=0.0,
                    base=-128 * jd, pattern=[[1, TCH]], channel_multiplier=-1,
                )
            # 1/SA: folds the oT fp8 scale into the softmax reciprocal
            nc.gpsimd.memset(ones_sb[:], 1.0 / SA)

            # per (tq-half, local head) AllToAll bounce buffers, fp8 hi/lo
            a2a_in = [[dramp.tile([NCORES, 128, 2, TQ], fp8,
                                  name=f"a2a_in{p}{h}") for h in range(HL)]
                      for p in range(2)]
            a2a_out = [[dramp.tile([NCORES, 128, 2, TQ], fp8,
                                   name=f"a2a_out{p}{h}") for h in range(HL)]
                       for p in range(2)]

            with tc.tile_pool(name="qkv", bufs=1) as qkvp:
                qT = [[qkvp.tile([128, T], bf16, name=f"qT_{hl}_{b}")
                       for b in range(B)] for hl in range(HL)]
                kT = [[qkvp.tile([128, T], bf16, name=f"kT_{hl}_{b}")
                       for b in range(B)] for hl in range(HL)]
                V = [qkvp.tile([128, TB, 2 * HD], bf16, name=f"V_{b}")
                     for b in range(B)]

                # -------- Phase 1: QKV projection + RoPE ------------------
                with (
                    tc.tile_pool(name="wqk", bufs=1) as wqkp,
                    tc.tile_pool(name="xin", bufs=2) as xp,
                    tc.tile_pool(name="ps_qk", bufs=4, space="PSUM") as psqk,
                    tc.tile_pool(name="ps_v", bufs=4, space="PSUM") as psv,
                    tc.tile_pool(name="rope", bufs=3) as ropep,
                ):
                    wqk_sb = wqkp.tile([128, CB, 2, 6 * HD], fp8,
                                       name="wqk_sb")
                    # startup: round-robin (wqk cb, x cb) bundles over all
                    # three DMA queues so MM(cb-pair) can start the moment
                    # its weight+activation blocks land
                    q3 = [nc.sync, nc.scalar, nc.gpsimd]

                    def rope_emit(ps, f, b, t0):
                        qraw = ropep.tile([128, TCH], bf16, tag="qraw",
                                          name=f"qraw_{b}_{t0}_{f}")
                        # fold out the fp8 scales while leaving PSUM
                        nc.scalar.mul(qraw[:], ps[:], QKV_DESC)
                        rot = ropep.tile([128, TCH], bf16, tag="rot",
                                         name=f"rot_{b}_{t0}_{f}")
                        nc.vector.stream_shuffle(rot[:], qraw[:],
                                                 mask=_SHUF)
                        t1 = ropep.tile([128, TCH], bf16, tag="t1",
                                        name=f"t1_{b}_{t0}_{f}")
                        nc.vector.tensor_mul(
                            t1[:], qraw[:], cos_sb[:, t0:t0 + TCH])
                        nc.vector.tensor_mul(
                            rot[:], rot[:], sin_sb[:, t0:t0 + TCH])
                        dest = (qT if f < 2 else kT)[f % 2][b]
                        nc.vector.tensor_add(
                            dest[:, t0:t0 + TCH], t1[:], rot[:])

                    def qk_mms(ps, f, xsl2, j, first, last, skip=False):
                        """3-term DoubleRow products for cb pair j, feature
                        tile f (q_h0,q_h1,k_h0,k_h1)."""
                        fsl = slice(f * 128, (f + 1) * 128)
                        terms = ((0, 0), (0, 1), (1, 0))  # (w,x) hi/lo
                        for ti, (a, bb) in enumerate(terms):
                            nc.tensor.matmul(
                                ps[:],
                                lhsT=wqk_sb[:, 2 * j:2 * j + 2, a, fsl],
                                rhs=xsl2(j, bb),
                                start=(first and ti == 0),
                                stop=(last and ti == 2),
                                perf_mode=DR, skip_group_check=skip)

                    def v_mms(pv, tb, xpan, h0, j, first, last, skip=False):
                        tsl = slice(h0 + tb * 128, h0 + (tb + 1) * 128)
                        terms = ((0, 0), (1, 0), (0, 1))  # (x,w) hi/lo
                        for ti, (bb, a) in enumerate(terms):
                            nc.tensor.matmul(
                                pv[:],
                                lhsT=xpan[:, 2 * j:2 * j + 2, bb, tsl],
                                rhs=wqk_sb[:, 2 * j:2 * j + 2, a,
                                           4 * 128:6 * 128],
                                start=(first and ti == 0),
                                stop=(last and ti == 2),
                                perf_mode=DR, skip_group_check=skip)

                    for pan in range(NPAN):
                        b, pc = divmod(pan, T // PAN)
                        xpan = xp.tile([128, CB, 2, PAN], fp8, tag="xpan",
                                       name=f"xpan_{pan}")
                        for cb in range(CB):
                            if pan == 0:
                                q3[cb % 3].dma_start(wqk_sb[:, cb, :, :],
                                                     wqkT_v[:, cb, :, :])
                            if pan <= 2:  # gpsimd free until the first A2A
                                eng = q3[cb % 3]
                            else:
                                eng = nc.sync if cb % 2 == 0 else nc.scalar
                            eng.dma_start(
                                xpan[:, cb, :, :],
                                xT_v[:, cb, pan, :, :])
                        if pan == 0:
                            # behind the critical first weight/x loads
                            nc.sync.dma_start(cos_sb[:], cosT_d[:, :])
                            nc.scalar.dma_start(sin_sb[:], sinS_d[:, :])

                        for half in range(PAN // TCH):
                            cc = pc * (PAN // TCH) + half
                            t0 = cc * TCH
                            h0 = half * TCH

                            def xsl2(j, bb):
                                return xpan[:, 2 * j:2 * j + 2, bb,
                                            h0:h0 + TCH]

                            if pan == 0 and half == 0:
                                # pair-outer warm start: consume each cb pair
                                # the moment its DMA lands (8 psum groups)
                                ps_f = [psqk.tile([128, TCH], f32, tag="qk",
                                                  name=f"psqk_0_{f}")
                                        for f in range(4)]
                                pv_t = [psv.tile([128, 2 * HD], f32, tag="v",
                                                 name=f"psv_0_{tb}")
                                        for tb in range(4)]
                                for j in range(NJP):
                                    for f in range(4):
                                        qk_mms(ps_f[f], f, xsl2, j,
                                               j == 0, j == NJP - 1,
                                               skip=True)
                                    for tb in range(4):
                                        v_mms(pv_t[tb], tb, xpan, h0, j,
                                              j == 0, j == NJP - 1,
                                              skip=True)
                                for f in range(4):
                                    rope_emit(ps_f[f], f, b, t0)
                                for tb in range(4):
                                    nc.scalar.mul(
                                        V[b][:, cc * 4 + tb, :], pv_t[tb][:],
                                        QKV_DESC)
                                continue

                            for f in range(4):  # q_h0 q_h1 k_h0 k_h1
                                ps = psqk.tile([128, TCH], f32, tag="qk",
                                               name=f"psqk_{pan}_{half}_{f}")
                                for j in range(NJP):
                                    qk_mms(ps, f, xsl2, j,
                                           j == 0, j == NJP - 1)
                                rope_emit(ps, f, b, t0)

                            for tb in range(4):  # v
                                pv = psv.tile([128, 2 * HD], f32, tag="v",
                                              name=f"psv_{pan}_{half}_{tb}")
                                for j in range(NJP):
                                    v_mms(pv, tb, xpan, h0, j,
                                          j == 0, j == NJP - 1)
                                nc.scalar.mul(V[b][:, cc * 4 + tb, :],
                                              pv[:], QKV_DESC)

                # -------- Phase 2: attention + resharding -----------------
                attnall_t = []
                wout_pre = {}
                with (
                    tc.tile_pool(name="attn", bufs=2) as attnp,
                    tc.tile_pool(name="wout", bufs=2) as woutp,
                ):
                  # prefetch first two out-proj weight tiles on the (now
                  # idle) scalar+sync queues, ahead of the exp stream
                  for oc in range(2):
                      w = woutp.tile([128, CB, 2, TCH], fp8, tag="wout",
                                     name=f"wout_0_{oc}")
                      nc.scalar.dma_start(
                          w[:, 0:CB // 2, :, :],
                          woutT_v[:, 0:CB // 2, oc, :, :])
                      nc.sync.dma_start(
                          w[:, CB // 2:CB, :, :],
                          woutT_v[:, CB // 2:CB, oc, :, :])
                      wout_pre[(0, oc)] = w
                  with (
                    tc.tile_pool(name="ps_st", bufs=2, space="PSUM") as psst,
                    tc.tile_pool(name="ps_acc", bufs=2, space="PSUM") as psacc,
                    tc.tile_pool(name="ps_on", bufs=2, space="PSUM") as psones,
                    tc.tile_pool(name="pexp", bufs=6) as pexpp,
                    tc.tile_pool(name="pacc", bufs=2) as accp,
                    tc.tile_pool(name="onorm", bufs=3) as onp,
                  ):
                    for part in range(2):
                        # layout [128, i(core), hl, t] == attnallT c order
                        ahi = attnp.tile([128, CB // 2, HL, TQ], fp8,
                                         tag="ahi", name=f"ahi_{part}")
                        alo = attnp.tile([128, CB // 2, HL, TQ], fp8,
                                         tag="alo", name=f"alo_{part}")
                        attnall_t.append((ahi, alo))
                        for hl in range(HL):
                            for tqc in range(part, CHB, 2):
                                for b in range(B):
                                    _attn_chunk(
                                        nc, mybir, psst, psacc, psones,
                                        pexpp, accp, onp,
                                        qT, kT, V, mask_sb, ones_sb,
                                        a2a_in[part][hl], b, hl, tqc,
                                        SCALE, Exp, f32, bf16, fp8)
                            # reshard this (half, head) while the rest of
                            # attention / the output projection runs
                            nc.gpsimd.collective_compute(
                                "AllToAll", mybir.AluOpType.bypass,
                                replica_groups=[list(range(NCORES))],
                                ins=[a2a_in[part][hl].opt()],
                                outs=[a2a_out[part][hl].opt()],
                            )
                            # post-collective load on gpsimd: idle between
                            # triggers, and the collectives serialize on the
                            # CC engine anyway, so this delays nothing
                            nc.gpsimd.dma_start(
                                ahi[:, :, hl, :],
                                a2a_out[part][hl][:, :, 0, :].rearrange(
                                    "i p t -> p i t"))
                            nc.gpsimd.dma_start(
                                alo[:, :, hl, :],
                                a2a_out[part][hl][:, :, 1, :].rearrange(
                                    "i p t -> p i t"))

                  # -------- Phase 3: output projection --------------------
                  with (
                    tc.tile_pool(name="ps_out", bufs=2, space="PSUM") as pso,
                    tc.tile_pool(name="o3", bufs=3) as o3p,
                  ):
                    last_mm = None
                    first_mm_p1 = None
                    for part in range(2):
                        ahi, alo = attnall_t[part]
                        for oc in range(4):
                            if (part, oc) in wout_pre:
                                w = wout_pre[(part, oc)]
                            else:
                                w = woutp.tile([128, CB, 2, TCH], fp8,
                                               tag="wout",
                                               name=f"wout_{part}_{oc}")
                                nc.scalar.dma_start(
                                    w[:, 0:CB // 2, :, :],
                                    woutT_v[:, 0:CB // 2, oc, :, :])
                                nc.sync.dma_start(
                                    w[:, CB // 2:CB, :, :],
                                    woutT_v[:, CB // 2:CB, oc, :, :])
                            for tb in range(TQ // 128):
                                po = pso.tile([128, TCH], f32, tag="out",
                                              name=f"po_{part}_{oc}_{tb}")
                                tsl = slice(tb * 128, (tb + 1) * 128)
                                i = 0
                                for j in range(NJP):
                                    # (attn, wout) hi/lo 3-term
                                    for (aa, ww) in ((0, 0), (0, 1), (1, 0)):
                                        mm = nc.tensor.matmul(
                                            po[:],
                                            lhsT=(ahi if aa == 0 else alo)[
                                                :, j, :, tsl],
                                            rhs=w[:, 2 * j:2 * j + 2, ww, :],
                                            start=(i == 0),
                                            stop=(i == 3 * NJP - 1),
                                            perf_mode=DR)
                                        i += 1
                                    if part == 1 and first_mm_p1 is None:
                                        first_mm_p1 = mm
                                    if part == 0:
                                        last_mm = mm
                                ot = o3p.tile([128, TCH], f32, tag="o3",
                                              name=f"ot_{part}_{oc}_{tb}")
                                # DVE copy w/ fp8 scale fold-out: ACT stays
                                # free for exp tails
                                nc.vector.tensor_scalar_mul(ot[:], po[:],
                                                            OUT_DESC)
                                # scalar-only: sync must stay clear for the
                                # last attention group's oT stores, else the
                                # final A2A fires late and out-proj stalls
                                nc.scalar.dma_start(
                                    out_d[part * TQ + tb * 128:
                                          part * TQ + (tb + 1) * 128,
                                          oc * TCH:(oc + 1) * TCH],
                                    ot[:])
                    # keep the two out-proj halves in emission order on the
                    # PE so part 1 (gated on the later collectives) cannot
                    # starve part 0's remaining matmuls
                    tile.add_dep_helper(
                        first_mm_p1.ins, last_mm.ins, sync=False,
                        reason="outproj part order")

    nc.compile()
    return nc


def _attn_chunk(nc, mybir, psst, psacc, psones, pexpp, accp, onp, qT, kT, V,
                mask_sb, ones_sb, a2a_in_ph, b, hl, tqc, SCALE, Exp, f32,
                bf16, fp8):
    """One (batch, head, 512-query-chunk) of causal attention."""
    ntk = (tqc + 1) * (TCH // 128)
    npair = ntk // 2
    q_sl = qT[hl][b][:, tqc * TCH:(tqc + 1) * TCH]
    av = psacc.tile([128, TCH], f32, tag="acc", name=f"av_{b}_{hl}_{tqc}")
    # denominator partials, one 512-wide lane-sum strip per pair half
    acc2 = accp.tile([128, 2, TCH], bf16, tag="pacc",
                     name=f"pa_{b}_{hl}_{tqc}")
    accf = accp.tile([128, TCH], bf16, tag="paccf",
                     name=f"pf_{b}_{hl}_{tqc}")
    pexp_t = {}
    acc_init = [True, True]

    def col0(j):
        """First valid tq column for tk-block j (causal: tq >= tk)."""
        jd = j - (TCH // 128) * tqc
        return 128 * jd if jd > 0 else 0

    def emit_pair(p):
        """scores -> exp -> mask for two 128-token tk blocks."""
        st = psst.tile([128, 2, TCH], f32, tag="st",
                       name=f"st_{b}_{hl}_{tqc}_{p}")
        pe = pexpp.tile([128, 2, TCH], bf16, tag="pexp",
                        name=f"pe_{b}_{hl}_{tqc}_{p}")
        for jj in range(2):
            j = 2 * p + jj
            c0 = col0(j)
            nc.tensor.matmul(
                st[:, jj, c0:TCH],
                lhsT=kT[hl][b][:, j * 128:(j + 1) * 128],
                rhs=q_sl[:, c0:TCH], start=True, stop=True)
        if col0(2 * p) == 0 and col0(2 * p + 1) == 0:
            nc.scalar.activation(pe[:], st[:], Exp, scale=SCALE)
        else:
            for jj in range(2):
                c0 = col0(2 * p + jj)
                nc.scalar.activation(
                    pe[:, jj, c0:TCH], st[:, jj, c0:TCH], Exp, scale=SCALE)
        for jj in range(2):
            j = 2 * p + jj
            jd = j - (TCH // 128) * tqc
            if jd >= 0:
                # causal mask: only the 128-col diagonal band is partial
                c0 = col0(j)
                nc.vector.tensor_mul(
                    pe[:, jj, c0:c0 + 128], pe[:, jj, c0:c0 + 128],
                    mask_sb[:, jd, c0:c0 + 128])
        pexp_t[p] = pe

    PIPEP = 2  # score/exp pairs emitted ahead of their AV consumption
    for p in range(min(PIPEP, npair)):
        emit_pair(p)
    for p in range(npair):
        if p + PIPEP < npair:
            emit_pair(p + PIPEP)
        pe = pexp_t.pop(p)
        for jj in range(2):
            j = 2 * p + jj
            c0 = col0(j)
            first = j == 0  # always full width: sets has_written everywhere
            last = j == ntk - 1
            nc.tensor.matmul(
                av[:, c0:TCH],
                lhsT=V[b][:, j, hl * 128:(hl + 1) * 128],
                rhs=pe[:, jj, c0:TCH],
                start=first, stop=last, skip_group_check=True)
        # softmax-denominator partials on the DVE (per tk-lane, per half)
        if col0(2 * p) == 0 and col0(2 * p + 1) == 0:
            if acc_init[0]:
                nc.vector.tensor_scalar_add(acc2[:], pe[:], 0.0)
                acc_init[0] = acc_init[1] = False
            else:
                nc.vector.tensor_add(acc2[:], acc2[:], pe[:])
        else:
            for jj in range(2):
                c0 = col0(2 * p + jj)
                dst = acc2[:, jj, c0:TCH]
                src = pe[:, jj, c0:TCH]
                if acc_init[jj]:
                    nc.vector.tensor_scalar_add(dst, src, 0.0)
                    acc_init[jj] = False
                else:
                    nc.vector.tensor_add(dst, dst, src)

    # fold halves, then one small ones-matmul for the partition sum
    if tqc == 0:  # acc2[:,1,0:128] never written (tk block 1 starts at 128)
        nc.vector.tensor_scalar_add(accf[:, 0:128], acc2[:, 0, 0:128], 0.0)
        nc.vector.tensor_add(accf[:, 128:], acc2[:, 0, 128:],
                             acc2[:, 1, 128:])
    else:
        nc.vector.tensor_add(accf[:], acc2[:, 0, :], acc2[:, 1, :])
    ones_ps = psones.tile([128, TCH], f32, tag="ones",
                          name=f"on_{b}_{hl}_{tqc}")
    # ones_sb holds 1/SA, so recip = SA/den and oT lands pre-scaled for fp8
    nc.tensor.matmul(ones_ps[:], lhsT=ones_sb[:], rhs=accf[:],
                     start=True, stop=True)
    recip = onp.tile([128, TCH], f32, tag="recip", name=f"rc_{b}_{hl}_{tqc}")
    nc.vector.reciprocal_approx_fast(recip[:], ones_ps[:])
    oT = onp.tile([128, TCH], bf16, tag="oT", name=f"oT_{b}_{hl}_{tqc}")
    nc.vector.tensor_mul(oT[:], av[:], recip[:])
    oT_hi = onp.tile([128, TCH], fp8, tag="oT_hi",
                     name=f"oTh_{b}_{hl}_{tqc}")
    nc.vector.tensor_copy(oT_hi[:], oT[:])
    oT_lo = onp.tile([128, TCH], fp8, tag="oT_lo",
                     name=f"oTl_{b}_{hl}_{tqc}")
    nc.vector.tensor_sub(oT_lo[:], oT[:], oT_hi[:])
    dj = b * 2 + tqc // 2
    nc.sync.dma_start(a2a_in_ph[dj, :, 0, :], oT_hi[:])
    nc.sync.dma_start(a2a_in_ph[dj, :, 1, :], oT_lo[:])


def _split8(a, scale):
    """fp8 hi/lo split with a shared power-of-2 scale. Returns fp8 arrays."""
    s = (a.astype(np.float32) * scale)
    hi = s.astype(E4M3)
    lo = (s - hi.astype(np.float32)).astype(E4M3)
    return hi, lo


def prep_inputs(x, cos, sin, w_qkv, w_out, T=2048):
    """Host-side shard/layout/quantize prep. Returns in_maps for 8 cores."""
    TOK = B * T
    xT = np.ascontiguousarray(x.reshape(TOK, D).T)
    xh, xl = _split8(xT, SX)
    # rows: [pan, hi/lo, t]
    xT2 = np.empty((D, TOK // PAN, 2, PAN), E4M3)
    xT2[:, :, 0, :] = xh.reshape(D, TOK // PAN, PAN)
    xT2[:, :, 1, :] = xl.reshape(D, TOK // PAN, PAN)
    xT2 = xT2.reshape(D, 2 * TOK)

    cosT = np.ascontiguousarray(cos.T[_PERM, :]).astype(BF16)
    sinS = np.ascontiguousarray(sin.T[_PERM, :] * _SIGN[:, None]).astype(BF16)

    woutT = np.ascontiguousarray(w_out.T)
    wh, wl = _split8(woutT, SW)
    woutT2 = np.empty((D, 4, 2, TCH), E4M3)
    wh = wh.reshape(D, 4, TCH)
    wl = wl.reshape(D, 4, TCH)
    woutT2[:, :, 0, :] = wh
    woutT2[:, :, 1, :] = wl
    woutT2 = woutT2.reshape(D, 2 * D)

    in_maps = []
    for c in range(NCORES):
        rows = []
        for sec in range(2):  # q, k (perm'd)
            for hl in range(HL):
                h = 2 * c + hl
                w = w_qkv[sec * D + h * HD:sec * D + (h + 1) * HD, :]
                rows.append(w[_PERM, :])
        for hl in range(HL):  # v natural
            h = 2 * c + hl
            rows.append(w_qkv[2 * D + h * HD:2 * D + (h + 1) * HD, :])
        wqkT = np.ascontiguousarray(np.concatenate(rows, 0).T)
        qh, ql = _split8(wqkT, SW)
        wqkT2 = np.empty((D, 2, 6 * HD), E4M3)
        wqkT2[:, 0, :] = qh
        wqkT2[:, 1, :] = ql
        wqkT2 = wqkT2.reshape(D, 2 * 6 * HD)
        in_maps.append({"xT2": xT2, "wqkT2": wqkT2, "woutT2": woutT2,
                        "cosT": cosT, "sinS": sinS})
    return in_maps


_NC_CACHE = {}


def _get_nc(T=2048):
    if T not in _NC_CACHE:
        _NC_CACHE[T] = build_nc(T)
    return _NC_CACHE[T]


def kernel(x, cos, sin, w_qkv, w_out):
    import concourse.bass_utils as bass_utils

    T = x.shape[1]
    x = np.asarray(x, np.float32)
    cos = np.asarray(cos, np.float32)
    sin = np.asarray(sin, np.float32)
    w_qkv = np.asarray(w_qkv, np.float32)
    w_out = np.asarray(w_out, np.float32)

    nc = _get_nc(T)
    in_maps = prep_inputs(x, cos, sin, w_qkv, w_out, T)
    res = bass_utils.run_bass_kernel_spmd(nc, in_maps,
                                          core_ids=list(range(NCORES)))
    THALF = T // 2
    full = np.empty((B, T, D), np.float32)
    for j in range(NCORES):
        b, hf = divmod(j, 2)
        full[b, hf * THALF:(hf + 1) * THALF, :] = res.results[j]["out"]
    return full


# revision 9
# speedup vs baseline: 1.3518x; 1.3518x over previous
"""Causal self-attention (B=4, T=2048, D=2048, H=16, HD=128) on 8 Trainium2
NeuronCores.

Sharding: Megatron-style tensor parallel over heads for QKV projection +
attention (2 heads per core), then on-device AllToAlls reshard from
head-parallel to token-parallel (core j owns tokens of batch j//2, half j%2)
for the output projection.  Host only slices/transposes weights, replicates
activations, and concatenates the 8 output shards.

Device layouts (all matmul operands bf16, fp32 PSUM accumulation):
  xT    [D, B*T]   x transposed (contraction dim on partitions)
  qT/kT [128, T]   per (local head, batch); d-order permuted so the RoPE
                   rotate-half partner sits 16 partitions away (within a
                   32-partition quadrant, reachable by DVE stream_shuffle).
                   Any consistent permutation of d leaves q.k unchanged.
  V     [T, 128]   natural d order (feeds AV matmul lhsT and out-proj order)
  S^T   [tk, tq]   scores transposed; no max-subtraction needed (logits ~
                   N(0,1), bounded ~ +-6, exp can't overflow).

The softmax denominator is accumulated on the DVE (acc[tk-lane, tq] +=
exp-tile per tk-block) and reduced across partitions by a single
128-column-stationary ones-matmul per 512-query chunk, instead of a
full-width ones-matmul per tk-block on the PE (saves ~9% PE columns; the
PE runs power-throttled at ~1.94 GHz so PE columns are the wall clock).

The attention loop runs tq-half 0 (even 512-token chunks) then half 1, with
one AllToAll per (half, head) issued as soon as that head's chunks finish —
all four collectives overlap the remaining attention / output projection.
The gpsimd queue carries ONLY the collective triggers; weight/activation
DMAs ride the sync + scalar HWDGE queues.
"""

import sys

for _p in ("/opt/trn_rl_repo", "/root/.axon_site/_ro/trn_rl_repo"):
    if _p not in sys.path:
        sys.path.insert(0, _p)

import numpy as np
import ml_dtypes

BF16 = ml_dtypes.bfloat16

B = 4
D = 2048
H = 16
HD = 128
NCORES = 8
HL = 2           # heads per core
CB = D // 128    # contraction blocks
TCH = 512        # token chunk (matmul moving free dim)
PAN = 1024       # x DMA panel (2 chunks; 2KB/partition DMA lines)


def _perm128():
    """Partition order for q/k head dims: quadrant g holds dims
    [16g,16g+16) (lo) then [64+16g, 64+16g+16) (hi), so the rotate-half
    partner of partition p is p+-16 (same 32-partition quadrant)."""
    perm = np.zeros(128, np.int64)
    for p in range(128):
        g, i = divmod(p, 32)
        perm[p] = g * 16 + i if i < 16 else 64 + g * 16 + (i - 16)
    return perm


_PERM = _perm128()
_SHUF = [(i + 16) % 32 for i in range(32)]  # out[i] = in[(i+16)%32]
_SIGN = np.where(np.arange(128) % 32 < 16, -1.0, 1.0).astype(np.float32)


def build_nc(T=2048):
    import concourse.bacc as bacc
    import concourse.tile as tile
    import concourse.mybir as mybir

    f32 = mybir.dt.float32
    bf16 = mybir.dt.bfloat16
    TOK = B * T
    THALF = T // 2
    TQ = THALF // 2           # tokens per (core, a2a part)
    NPAN = TOK // PAN         # x panels total
    CHB = T // TCH            # token chunks per batch
    TB = T // 128             # 128-token blocks per batch
    SCALE = float(HD) ** -0.5
    Exp = mybir.ActivationFunctionType.Exp

    assert TQ == TCH, "A2A split layout assumes T == 2048"
    nc = bacc.Bacc("TRN2", target_bir_lowering=False, debug=False,
                   num_devices=NCORES)

    xT_d = nc.dram_tensor("xT", [D, TOK], bf16, kind="ExternalInput")
    wqkT_d = nc.dram_tensor("wqkT", [D, 6 * HD], bf16, kind="ExternalInput")
    woutT_d = nc.dram_tensor("woutT", [D, D], bf16, kind="ExternalInput")
    cosT_d = nc.dram_tensor("cosT", [HD, T], bf16, kind="ExternalInput")
    sinS_d = nc.dram_tensor("sinS", [HD, T], bf16, kind="ExternalInput")
    out_d = nc.dram_tensor("out", [THALF, D], f32, kind="ExternalOutput")

    xT_v = xT_d.ap().rearrange("(cb p) t -> p cb t", p=128)
    wqkT_v = wqkT_d.ap().rearrange("(cb p) f -> p cb f", p=128)
    woutT_v = woutT_d.ap().rearrange("(cb p) o -> p cb o", p=128)

    with tile.TileContext(nc) as tc:
        with (
            tc.tile_pool(name="const", bufs=1) as constp,
            tc.tile_pool(name="dram", bufs=1, space="DRAM") as dramp,
        ):
            cos_sb = constp.tile([128, T], bf16, name="cos_sb")
            sin_sb = constp.tile([128, T], bf16, name="sin_sb")
            mask_sb = constp.tile([128, 4, TCH], bf16, name="mask_sb")
            ones_sb = constp.tile([128, 128], bf16, name="ones_sb")
            nc.gpsimd.memset(mask_sb[:], 1.0)
            for jd in range(4):
                # keep 1.0 where  tq_rel - tk_rel - 128*jd >= 0  else 0
                nc.gpsimd.affine_select(
                    out=mask_sb[:, jd, :], in_=mask_sb[:, jd, :],
                    compare_op=mybir.AluOpType.is_ge, fill=0.0,
                    base=-128 * jd, pattern=[[1, TCH]], channel_multiplier=-1,
                )
            nc.gpsimd.memset(ones_sb[:], 1.0)

            # per (tq-half, local head) AllToAll bounce buffers
            a2a_in = [[dramp.tile([NCORES, 128, TQ], bf16,
                                  name=f"a2a_in{p}{h}") for h in range(HL)]
                      for p in range(2)]
            a2a_out = [[dramp.tile([NCORES, 128, TQ], bf16,
                                   name=f"a2a_out{p}{h}") for h in range(HL)]
                       for p in range(2)]

            with tc.tile_pool(name="qkv", bufs=1) as qkvp:
                qT = [[qkvp.tile([128, T], bf16, name=f"qT_{hl}_{b}")
                       for b in range(B)] for hl in range(HL)]
                kT = [[qkvp.tile([128, T], bf16, name=f"kT_{hl}_{b}")
                       for b in range(B)] for hl in range(HL)]
                V = [qkvp.tile([128, TB, 2 * HD], bf16, name=f"V_{b}")
                     for b in range(B)]

                # -------- Phase 1: QKV projection + RoPE ------------------
                with (
                    tc.tile_pool(name="wqk", bufs=1) as wqkp,
                    tc.tile_pool(name="xin", bufs=2) as xp,
                    tc.tile_pool(name="ps_qk", bufs=4, space="PSUM") as psqk,
                    tc.tile_pool(name="ps_v", bufs=4, space="PSUM") as psv,
                    tc.tile_pool(name="rope", bufs=3) as ropep,
                ):
                    wqk_sb = wqkp.tile([128, CB, 6 * HD], bf16,
                                       name="wqk_sb")
                    # startup: round-robin (wqk cb, x cb) bundles over all
                    # three DMA queues so MM(cb) can start the moment its
                    # weight+activation blocks land (gpsimd queue is free
                    # until the first collective, ~450us in)
                    q3 = [nc.sync, nc.scalar, nc.gpsimd]

                    def rope_emit(ps, f, b, t0):
                        qraw = ropep.tile([128, TCH], bf16, tag="qraw",
                                          name=f"qraw_{b}_{t0}_{f}")
                        nc.scalar.copy(qraw[:], ps[:])
                        rot = ropep.tile([128, TCH], bf16, tag="rot",
                                         name=f"rot_{b}_{t0}_{f}")
                        nc.vector.stream_shuffle(rot[:], qraw[:],
                                                 mask=_SHUF)
                        t1 = ropep.tile([128, TCH], bf16, tag="t1",
                                        name=f"t1_{b}_{t0}_{f}")
                        nc.vector.tensor_mul(
                            t1[:], qraw[:], cos_sb[:, t0:t0 + TCH])
                        nc.vector.tensor_mul(
                            rot[:], rot[:], sin_sb[:, t0:t0 + TCH])
                        dest = (qT if f < 2 else kT)[f % 2][b]
                        nc.vector.tensor_add(
                            dest[:, t0:t0 + TCH], t1[:], rot[:])

                    # rope tables: first 2 chunks' columns ride at the head
                    # of the (otherwise idle) gpsimd queue so chunk 0/1's
                    # rope_emit never waits; the tail rides behind pan 0.
                    nc.gpsimd.dma_start(cos_sb[:, 0:PAN], cosT_d[:, 0:PAN])
                    nc.gpsimd.dma_start(sin_sb[:, 0:PAN], sinS_d[:, 0:PAN])

                    for pan in range(NPAN):
                        b, pc = divmod(pan, T // PAN)
                        xpan = xp.tile([128, CB, PAN], bf16, tag="xpan",
                                       name=f"xpan_{pan}")
                        for cb in range(CB):
                            if pan == 0:
                                q3[cb % 3].dma_start(wqk_sb[:, cb, :],
                                                     wqkT_v[:, cb, :])
                            if pan <= 2:  # gpsimd free until the first A2A
                                eng = q3[cb % 3]
                            else:
                                eng = nc.sync if cb % 2 == 0 else nc.scalar
                            eng.dma_start(
                                xpan[:, cb, :],
                                xT_v[:, cb, pan * PAN:(pan + 1) * PAN])
                        if pan == 0:
                            nc.gpsimd.dma_start(cos_sb[:, PAN:T],
                                                cosT_d[:, PAN:T])
                            nc.gpsimd.dma_start(sin_sb[:, PAN:T],
                                                sinS_d[:, PAN:T])

                        for half in range(PAN // TCH):
                            cc = pc * (PAN // TCH) + half
                            t0 = cc * TCH
                            h0 = half * TCH

                            def xsl(cb):
                                return xpan[:, cb, h0:h0 + TCH]

                            if pan == 0 and half == 0:
                                # cb-outer warm start: consume each cb the
                                # moment its DMA lands (8 psum groups live)
                                ps_f = [psqk.tile([128, TCH], f32, tag="qk",
                                                  name=f"psqk_0_{f}")
                                        for f in range(4)]
                                pv_t = [psv.tile([128, 2 * HD], f32, tag="v",
                                                 name=f"psv_0_{tb}")
                                        for tb in range(4)]
                                for cb in range(CB):
                                    for f in range(4):
                                        nc.tensor.matmul(
                                            ps_f[f][:],
                                            lhsT=wqk_sb[:, cb,
                                                        f * 128:(f + 1) * 128],
                                            rhs=xsl(cb), start=(cb == 0),
                                            stop=(cb == CB - 1),
                                            skip_group_check=True)
                                    for tb in range(4):
                                        nc.tensor.matmul(
                                            pv_t[tb][:],
                                            lhsT=xpan[:, cb,
                                                      h0 + tb * 128:
                                                      h0 + (tb + 1) * 128],
                                            rhs=wqk_sb[:, cb,
                                                       4 * 128:6 * 128],
                                            start=(cb == 0),
                                            stop=(cb == CB - 1),
                                            skip_group_check=True)
                                for f in range(4):
                                    rope_emit(ps_f[f], f, b, t0)
                                for tb in range(4):
                                    nc.scalar.copy(
                                        V[b][:, cc * 4 + tb, :], pv_t[tb][:])
                                continue

                            for f in range(4):  # q_h0 q_h1 k_h0 k_h1
                                ps = psqk.tile([128, TCH], f32, tag="qk",
                                               name=f"psqk_{pan}_{half}_{f}")
                                for cb in range(CB):
                                    nc.tensor.matmul(
                                        ps[:],
                                        lhsT=wqk_sb[:, cb,
                                                    f * 128:(f + 1) * 128],
                                        rhs=xsl(cb),
                                        start=(cb == 0), stop=(cb == CB - 1))
                                rope_emit(ps, f, b, t0)

                            for tb in range(4):  # v
                                pv = psv.tile([128, 2 * HD], f32, tag="v",
                                              name=f"psv_{pan}_{half}_{tb}")
                                for cb in range(CB):
                                    nc.tensor.matmul(
                                        pv[:],
                                        lhsT=xpan[:, cb,
                                                  h0 + tb * 128:
                                                  h0 + (tb + 1) * 128],
                                        rhs=wqk_sb[:, cb, 4 * 128:6 * 128],
                                        start=(cb == 0), stop=(cb == CB - 1))
                                nc.scalar.copy(V[b][:, cc * 4 + tb, :],
                                               pv[:])

                # -------- Phase 2: attention + resharding -----------------
                attnall_t = []
                wout_pre = {}
                with (
                    tc.tile_pool(name="attn", bufs=2) as attnp,
                    tc.tile_pool(name="wout", bufs=2) as woutp,
                ):
                  # prefetch first two out-proj weight tiles on the (now
                  # idle) scalar+sync queues, ahead of the exp stream
                  for oc in range(2):
                      w = woutp.tile([128, CB, TCH], bf16, tag="wout",
                                     name=f"wout_0_{oc}")
                      nc.scalar.dma_start(
                          w[:, 0:CB // 2, :],
                          woutT_v[:, 0:CB // 2, oc * TCH:(oc + 1) * TCH])
                      nc.sync.dma_start(
                          w[:, CB // 2:CB, :],
                          woutT_v[:, CB // 2:CB, oc * TCH:(oc + 1) * TCH])
                      wout_pre[(0, oc)] = w
                  with (
                    tc.tile_pool(name="ps_st", bufs=2, space="PSUM") as psst,
                    tc.tile_pool(name="ps_acc", bufs=2, space="PSUM") as psacc,
                    tc.tile_pool(name="ps_on", bufs=2, space="PSUM") as psones,
                    tc.tile_pool(name="pexp", bufs=6) as pexpp,
                    tc.tile_pool(name="pacc", bufs=2) as accp,
                    tc.tile_pool(name="onorm", bufs=3) as onp,
                  ):
                    pend_ep = [None]  # deferred epilogue of the prev chunk

                    def run_pend():
                        if pend_ep[0] is not None:
                            pend_ep[0]()
                            pend_ep[0] = None

                    for part in range(2):
                        # layout [128, i(core), hl, t] == attnallT c order
                        attnall = attnp.tile([128, CB // 2, HL, TQ], bf16,
                                             tag="attnall",
                                             name=f"attnall_{part}")
                        attnall_t.append(attnall)
                        for hl in range(HL):
                            # b-outer: alternate small/large tq chunks so
                            # DVE/ACT epilogues hide under big-chunk PE time
                            for b in range(B):
                                for tqc in range(part, CHB, 2):
                                    pend_ep[0] = _attn_chunk(
                                        nc, mybir, psst, psacc, psones,
                                        pexpp, accp, onp,
                                        qT, kT, V, mask_sb, ones_sb,
                                        a2a_in[part][hl], b, hl, tqc,
                                        SCALE, Exp, f32, bf16, run_pend)
                            # flush the group's last epilogue before its A2A
                            run_pend()
                            # reshard this (half, head) while the rest of
                            # attention / the output projection runs
                            nc.gpsimd.collective_compute(
                                "AllToAll", mybir.AluOpType.bypass,
                                replica_groups=[list(range(NCORES))],
                                ins=[a2a_in[part][hl].opt()],
                                outs=[a2a_out[part][hl].opt()],
                            )
                            # post-collective load on gpsimd: idle between triggers,
                            # and the collectives serialize on the CC
                            # engine anyway, so this delays nothing
                            nc.gpsimd.dma_start(
                                attnall[:, :, hl, :],
                                a2a_out[part][hl].rearrange(
                                    "i p t -> p i t"))

                  # -------- Phase 3: output projection --------------------
                  with (
                    tc.tile_pool(name="ps_out", bufs=2, space="PSUM") as pso,
                    tc.tile_pool(name="o3", bufs=3) as o3p,
                  ):
                    last_mm = None
                    first_mm_p1 = None
                    for part in range(2):
                        attnall = attnall_t[part]
                        for oc in range(4):
                            if (part, oc) in wout_pre:
                                w = wout_pre[(part, oc)]
                            else:
                                w = woutp.tile([128, CB, TCH], bf16,
                                               tag="wout",
                                               name=f"wout_{part}_{oc}")
                                nc.scalar.dma_start(
                                    w[:, 0:CB // 2, :],
                                    woutT_v[:, 0:CB // 2,
                                            oc * TCH:(oc + 1) * TCH])
                                nc.sync.dma_start(
                                    w[:, CB // 2:CB, :],
                                    woutT_v[:, CB // 2:CB,
                                            oc * TCH:(oc + 1) * TCH])
                            for tb in range(TQ // 128):
                                po = pso.tile([128, TCH], f32, tag="out",
                                              name=f"po_{part}_{oc}_{tb}")
                                for cb in range(CB):
                                    mm = nc.tensor.matmul(
                                        po[:],
                                        lhsT=attnall[:, cb // 2, cb % 2,
                                                     tb * 128:(tb + 1) * 128],
                                        rhs=w[:, cb, :],
                                        start=(cb == 0),
                                        stop=(cb == CB - 1))
                                    if part == 1 and first_mm_p1 is None:
                                        first_mm_p1 = mm
                                    if part == 0:
                                        last_mm = mm
                                ot = o3p.tile([128, TCH], f32, tag="o3",
                                              name=f"ot_{part}_{oc}_{tb}")
                                # DVE copy: ACT stays free for exp tails
                                nc.vector.tensor_scalar_add(ot[:], po[:],
                                                            0.0)
                                # part 0 stores scalar-only: sync must stay
                                # clear for the last attention group's oT
                                # stores, else the final A2A fires late and
                                # out-proj stalls.  For part 1 the A2As are
                                # all done, so alternate queues to halve the
                                # tail drain after the last matmul.
                                if part == 0:
                                    st_eng = nc.scalar
                                else:
                                    st_eng = nc.scalar if tb % 2 == 0 \
                                        else nc.sync
                                st_eng.dma_start(
                                    out_d[part * TQ + tb * 128:
                                          part * TQ + (tb + 1) * 128,
                                          oc * TCH:(oc + 1) * TCH],
                                    ot[:])
                    # keep the two out-proj halves in emission order on the
                    # PE so part 1 (gated on the later collectives) cannot
                    # starve part 0's remaining matmuls
                    tile.add_dep_helper(
                        first_mm_p1.ins, last_mm.ins, sync=False,
                        reason="outproj part order")

    nc.compile()
    return nc


def _attn_chunk(nc, mybir, psst, psacc, psones, pexpp, accp, onp, qT, kT, V,
                mask_sb, ones_sb, a2a_in_ph, b, hl, tqc, SCALE, Exp, f32,
                bf16, run_pend):
    """One (batch, head, 512-query-chunk) of causal attention.

    Emits scores/exp/AV/denominator work and returns an epilogue closure
    (fold + ones-matmul + recip + oT + store).  The caller runs the
    epilogue only after the NEXT chunk's score pairs are on the PE queue,
    so the PE never stalls waiting for the DVE to finish the denominator
    (the previous ~0.6us bubble at every chunk boundary)."""
    ntk = (tqc + 1) * (TCH // 128)
    npair = ntk // 2
    q_sl = qT[hl][b][:, tqc * TCH:(tqc + 1) * TCH]
    av = psacc.tile([128, TCH], f32, tag="acc", name=f"av_{b}_{hl}_{tqc}")
    # denominator partials, one 512-wide lane-sum strip per pair half
    acc2 = accp.tile([128, 2, TCH], bf16, tag="pacc",
                     name=f"pa_{b}_{hl}_{tqc}")
    accf = accp.tile([128, TCH], bf16, tag="paccf",
                     name=f"pf_{b}_{hl}_{tqc}")
    pexp_t = {}
    acc_init = [True, True]

    def col0(j):
        """First valid tq column for tk-block j (causal: tq >= tk)."""
        jd = j - (TCH // 128) * tqc
        return 128 * jd if jd > 0 else 0

    def emit_pair(p):
        """scores -> exp -> mask for two 128-token tk blocks."""
        st = psst.tile([128, 2, TCH], f32, tag="st",
                       name=f"st_{b}_{hl}_{tqc}_{p}")
        pe = pexpp.tile([128, 2, TCH], bf16, tag="pexp",
                        name=f"pe_{b}_{hl}_{tqc}_{p}")
        for jj in range(2):
            j = 2 * p + jj
            c0 = col0(j)
            nc.tensor.matmul(
                st[:, jj, c0:TCH],
                lhsT=kT[hl][b][:, j * 128:(j + 1) * 128],
                rhs=q_sl[:, c0:TCH], start=True, stop=True)
        if col0(2 * p) == 0 and col0(2 * p + 1) == 0:
            nc.scalar.activation(pe[:], st[:], Exp, scale=SCALE)
        else:
            for jj in range(2):
                c0 = col0(2 * p + jj)
                nc.scalar.activation(
                    pe[:, jj, c0:TCH], st[:, jj, c0:TCH], Exp, scale=SCALE)
        for jj in range(2):
            j = 2 * p + jj
            jd = j - (TCH // 128) * tqc
            if jd >= 0:
                # causal mask: only the 128-col diagonal band is partial
                c0 = col0(j)
                nc.vector.tensor_mul(
                    pe[:, jj, c0:c0 + 128], pe[:, jj, c0:c0 + 128],
                    mask_sb[:, jd, c0:c0 + 128])
        pexp_t[p] = pe

    PIPEP = 2  # score/exp pairs emitted ahead of their AV consumption
    for p in range(min(PIPEP, npair)):
        emit_pair(p)
        # previous chunk's epilogue rides between our score pairs: by now
        # its DVE denominator folds have drained, so its ones-matmul and
        # oT production cost the PE nothing
        run_pend()
    for p in range(npair):
        if p + PIPEP < npair:
            emit_pair(p + PIPEP)
        pe = pexp_t.pop(p)
        for jj in range(2):
            j = 2 * p + jj
            c0 = col0(j)
            first = j == 0  # always full width: sets has_written everywhere
            last = j == ntk - 1
            nc.tensor.matmul(
                av[:, c0:TCH],
                lhsT=V[b][:, j, hl * 128:(hl + 1) * 128],
                rhs=pe[:, jj, c0:TCH],
                start=first, stop=last, skip_group_check=True)
        # softmax-denominator partials on the DVE (per tk-lane, per half)
        if col0(2 * p) == 0 and col0(2 * p + 1) == 0:
            if acc_init[0]:
                nc.vector.tensor_scalar_add(acc2[:], pe[:], 0.0)
                acc_init[0] = acc_init[1] = False
            else:
                nc.vector.tensor_add(acc2[:], acc2[:], pe[:])
        else:
            for jj in range(2):
                c0 = col0(2 * p + jj)
                dst = acc2[:, jj, c0:TCH]
                src = pe[:, jj, c0:TCH]
                if acc_init[jj]:
                    nc.vector.tensor_scalar_add(dst, src, 0.0)
                    acc_init[jj] = False
                else:
                    nc.vector.tensor_add(dst, dst, src)

    # fold halves on the DVE now (they chase the AV matmuls), but defer the
    # PE/DVE tail into an epilogue the caller schedules after the next
    # chunk's first score pairs
    if tqc == 0:  # acc2[:,1,0:128] never written (tk block 1 starts at 128)
        nc.vector.tensor_scalar_add(accf[:, 0:128], acc2[:, 0, 0:128], 0.0)
        nc.vector.tensor_add(accf[:, 128:], acc2[:, 0, 128:],
                             acc2[:, 1, 128:])
    else:
        nc.vector.tensor_add(accf[:], acc2[:, 0, :], acc2[:, 1, :])

    def epilogue():
        ones_ps = psones.tile([128, TCH], f32, tag="ones",
                              name=f"on_{b}_{hl}_{tqc}")
        nc.tensor.matmul(ones_ps[:], lhsT=ones_sb[:], rhs=accf[:],
                         start=True, stop=True)
        recip = onp.tile([128, TCH], f32, tag="recip",
                         name=f"rc_{b}_{hl}_{tqc}")
        nc.vector.reciprocal_approx_fast(recip[:], ones_ps[:])
        oT = onp.tile([128, TCH], bf16, tag="oT", name=f"oT_{b}_{hl}_{tqc}")
        nc.vector.tensor_mul(oT[:], av[:], recip[:])
        dj = b * 2 + tqc // 2
        nc.sync.dma_start(a2a_in_ph[dj, :, :], oT[:])

    return epilogue


def prep_inputs(x, cos, sin, w_qkv, w_out, T=2048):
    """Host-side shard/layout prep. Returns in_maps for the 8 cores."""
    TOK = B * T
    xT = np.ascontiguousarray(x.reshape(TOK, D).T).astype(BF16)
    cosT = np.ascontiguousarray(cos.T[_PERM, :]).astype(BF16)
    sinS = np.ascontiguousarray(sin.T[_PERM, :] * _SIGN[:, None]).astype(BF16)
    woutT = np.ascontiguousarray(w_out.T).astype(BF16)
    in_maps = []
    for c in range(NCORES):
        rows = []
        for sec in range(2):  # q, k (perm'd)
            for hl in range(HL):
                h = 2 * c + hl
                w = w_qkv[sec * D + h * HD:sec * D + (h + 1) * HD, :]
                rows.append(w[_PERM, :])
        for hl in range(HL):  # v natural
            h = 2 * c + hl
            rows.append(w_qkv[2 * D + h * HD:2 * D + (h + 1) * HD, :])
        wqkT = np.ascontiguousarray(np.concatenate(rows, 0).T).astype(BF16)
        in_maps.append({"xT": xT, "wqkT": wqkT, "woutT": woutT,
                        "cosT": cosT, "sinS": sinS})
    return in_maps


_NC_CACHE = {}


def _get_nc(T=2048):
    if T not in _NC_CACHE:
        _NC_CACHE[T] = build_nc(T)
    return _NC_CACHE[T]


def kernel(x, cos, sin, w_qkv, w_out):
    import concourse.bass_utils as bass_utils

    T = x.shape[1]
    x = np.asarray(x, np.float32)
    cos = np.asarray(cos, np.float32)
    sin = np.asarray(sin, np.float32)
    w_qkv = np.asarray(w_qkv, np.float32)
    w_out = np.asarray(w_out, np.float32)

    nc = _get_nc(T)
    in_maps = prep_inputs(x, cos, sin, w_qkv, w_out, T)
    res = bass_utils.run_bass_kernel_spmd(nc, in_maps,
                                          core_ids=list(range(NCORES)))
    THALF = T // 2
    full = np.empty((B, T, D), np.float32)
    for j in range(NCORES):
        b, hf = divmod(j, 2)
        full[b, hf * THALF:(hf + 1) * THALF, :] = res.results[j]["out"]
    return full



# revision 19
# speedup vs baseline: 1.3569x; 1.0038x over previous
"""Causal self-attention (B=4, T=2048, D=2048, H=16, HD=128) on 8 Trainium2
NeuronCores.

Sharding: Megatron-style tensor parallel over heads for QKV projection +
attention (2 heads per core), then on-device AllToAlls reshard from
head-parallel to token-parallel (core j owns tokens of batch j//2, half j%2)
for the output projection.  Host only slices/transposes weights, replicates
activations, and concatenates the 8 output shards.

Device layouts (all matmul operands bf16, fp32 PSUM accumulation):
  xT    [D, B*T]   x transposed (contraction dim on partitions)
  qT/kT [128, T]   per (local head, batch); d-order permuted so the RoPE
                   rotate-half partner sits 16 partitions away (within a
                   32-partition quadrant, reachable by DVE stream_shuffle).
                   Any consistent permutation of d leaves q.k unchanged.
  V     [T, 128]   natural d order (feeds AV matmul lhsT and out-proj order)
  S^T   [tk, tq]   scores transposed; no max-subtraction needed (logits ~
                   N(0,1), bounded ~ +-6, exp can't overflow).

The softmax denominator is accumulated on the DVE (acc[tk-lane, tq] +=
exp-tile per tk-block) and reduced across partitions by a single
128-column-stationary ones-matmul per 512-query chunk, instead of a
full-width ones-matmul per tk-block on the PE (saves ~9% PE columns; the
PE runs power-throttled at ~1.94 GHz so PE columns are the wall clock).

The attention loop runs tq-half 0 (even 512-token chunks) then half 1, with
one AllToAll per (half, head) issued as soon as that head's chunks finish —
all four collectives overlap the remaining attention / output projection.
The gpsimd queue carries ONLY the collective triggers; weight/activation
DMAs ride the sync + scalar HWDGE queues.
"""

import sys

for _p in ("/opt/trn_rl_repo", "/root/.axon_site/_ro/trn_rl_repo"):
    if _p not in sys.path:
        sys.path.insert(0, _p)

import numpy as np
import ml_dtypes

BF16 = ml_dtypes.bfloat16

B = 4
D = 2048
H = 16
HD = 128
NCORES = 8
HL = 2           # heads per core
CB = D // 128    # contraction blocks
TCH = 512        # token chunk (matmul moving free dim)
PAN = 1024       # x DMA panel (2 chunks; 2KB/partition DMA lines)


def _perm128():
    """Partition order for q/k head dims: quadrant g holds dims
    [16g,16g+16) (lo) then [64+16g, 64+16g+16) (hi), so the rotate-half
    partner of partition p is p+-16 (same 32-partition quadrant)."""
    perm = np.zeros(128, np.int64)
    for p in range(128):
        g, i = divmod(p, 32)
        perm[p] = g * 16 + i if i < 16 else 64 + g * 16 + (i - 16)
    return perm


_PERM = _perm128()
_SHUF = [(i + 16) % 32 for i in range(32)]  # out[i] = in[(i+16)%32]
_SIGN = np.where(np.arange(128) % 32 < 16, -1.0, 1.0).astype(np.float32)


def build_nc(T=2048):
    import concourse.bacc as bacc
    import concourse.tile as tile
    import concourse.mybir as mybir

    f32 = mybir.dt.float32
    bf16 = mybir.dt.bfloat16
    TOK = B * T
    THALF = T // 2
    TQ = THALF // 2           # tokens per (core, a2a part)
    NPAN = TOK // PAN         # x panels total
    CHB = T // TCH            # token chunks per batch
    TB = T // 128             # 128-token blocks per batch
    SCALE = float(HD) ** -0.5
    Exp = mybir.ActivationFunctionType.Exp

    assert TQ == TCH, "A2A split layout assumes T == 2048"
    nc = bacc.Bacc("TRN2", target_bir_lowering=False, debug=False,
                   num_devices=NCORES)

    xT_d = nc.dram_tensor("xT", [D, TOK], bf16, kind="ExternalInput")
    wqkT_d = nc.dram_tensor("wqkT", [D, 6 * HD], bf16, kind="ExternalInput")
    woutT_d = nc.dram_tensor("woutT", [D, D], bf16, kind="ExternalInput")
    cosT_d = nc.dram_tensor("cosT", [HD, T], bf16, kind="ExternalInput")
    sinS_d = nc.dram_tensor("sinS", [HD, T], bf16, kind="ExternalInput")
    out_d = nc.dram_tensor("out", [THALF, D], f32, kind="ExternalOutput")

    xT_v = xT_d.ap().rearrange("(cb p) t -> p cb t", p=128)
    wqkT_v = wqkT_d.ap().rearrange("(cb p) f -> p cb f", p=128)
    woutT_v = woutT_d.ap().rearrange("(cb p) o -> p cb o", p=128)

    with tile.TileContext(nc) as tc:
        with (
            tc.tile_pool(name="const", bufs=1) as constp,
            tc.tile_pool(name="dram", bufs=1, space="DRAM") as dramp,
        ):
            cos_sb = constp.tile([128, T], bf16, name="cos_sb")
            sin_sb = constp.tile([128, T], bf16, name="sin_sb")
            mask_sb = constp.tile([128, 4, TCH], bf16, name="mask_sb")
            ones_sb = constp.tile([128, 128], bf16, name="ones_sb")
            nc.gpsimd.memset(mask_sb[:], 1.0)
            for jd in range(4):
                # keep 1.0 where  tq_rel - tk_rel - 128*jd >= 0  else 0
                nc.gpsimd.affine_select(
                    out=mask_sb[:, jd, :], in_=mask_sb[:, jd, :],
                    compare_op=mybir.AluOpType.is_ge, fill=0.0,
                    base=-128 * jd, pattern=[[1, TCH]], channel_multiplier=-1,
                )
            nc.gpsimd.memset(ones_sb[:], 1.0)

            # per (tq-half, local head) AllToAll bounce buffers
            a2a_in = [[dramp.tile([NCORES, 128, TQ], bf16,
                                  name=f"a2a_in{p}{h}") for h in range(HL)]
                      for p in range(2)]
            a2a_out = [[dramp.tile([NCORES, 128, TQ], bf16,
                                   name=f"a2a_out{p}{h}") for h in range(HL)]
                       for p in range(2)]

            with tc.tile_pool(name="qkv", bufs=1) as qkvp:
                qT = [[qkvp.tile([128, T], bf16, name=f"qT_{hl}_{b}")
                       for b in range(B)] for hl in range(HL)]
                kT = [[qkvp.tile([128, T], bf16, name=f"kT_{hl}_{b}")
                       for b in range(B)] for hl in range(HL)]
                V = [qkvp.tile([128, TB, 2 * HD], bf16, name=f"V_{b}")
                     for b in range(B)]

                # -------- Phase 1: QKV projection + RoPE ------------------
                with (
                    tc.tile_pool(name="wqk", bufs=1) as wqkp,
                    tc.tile_pool(name="xin", bufs=2) as xp,
                    tc.tile_pool(name="ps_qk", bufs=4, space="PSUM") as psqk,
                    tc.tile_pool(name="ps_v", bufs=4, space="PSUM") as psv,
                    tc.tile_pool(name="rope", bufs=3) as ropep,
                ):
                    wqk_sb = wqkp.tile([128, CB, 6 * HD], bf16,
                                       name="wqk_sb")
                    # startup: round-robin (wqk cb, x cb) bundles over all
                    # three DMA queues so MM(cb) can start the moment its
                    # weight+activation blocks land (gpsimd queue is free
                    # until the first collective, ~450us in)
                    q3 = [nc.sync, nc.scalar, nc.gpsimd]

                    def rope_emit(ps, f, b, t0, dve_copy=False):
                        qraw = ropep.tile([128, TCH], bf16, tag="qraw",
                                          name=f"qraw_{b}_{t0}_{f}")
                        if dve_copy:
                            # last chunk: keep ACT clear so the first
                            # attention exps aren't stuck behind its copies
                            nc.vector.tensor_scalar_add(qraw[:], ps[:], 0.0)
                        else:
                            nc.scalar.copy(qraw[:], ps[:])
                        rot = ropep.tile([128, TCH], bf16, tag="rot",
                                         name=f"rot_{b}_{t0}_{f}")
                        nc.vector.stream_shuffle(rot[:], qraw[:],
                                                 mask=_SHUF)
                        t1 = ropep.tile([128, TCH], bf16, tag="t1",
                                        name=f"t1_{b}_{t0}_{f}")
                        nc.vector.tensor_mul(
                            t1[:], qraw[:], cos_sb[:, t0:t0 + TCH])
                        nc.vector.tensor_mul(
                            rot[:], rot[:], sin_sb[:, t0:t0 + TCH])
                        dest = (qT if f < 2 else kT)[f % 2][b]
                        nc.vector.tensor_add(
                            dest[:, t0:t0 + TCH], t1[:], rot[:])

                    # rope tables: chunk 0's columns ride at the head of the
                    # (otherwise idle) gpsimd queue -- 256KB, small enough
                    # not to delay the warm-start cb loads behind them --
                    # so chunk 0's rope_emit never waits on cos/sin.
                    nc.gpsimd.dma_start(cos_sb[:, 0:TCH], cosT_d[:, 0:TCH])
                    nc.gpsimd.dma_start(sin_sb[:, 0:TCH], sinS_d[:, 0:TCH])

                    for pan in range(NPAN):
                        b, pc = divmod(pan, T // PAN)
                        xpan = xp.tile([128, CB, PAN], bf16, tag="xpan",
                                       name=f"xpan_{pan}")
                        for cb in range(CB):
                            if pan == 0:
                                q3[cb % 3].dma_start(wqk_sb[:, cb, :],
                                                     wqkT_v[:, cb, :])
                            if pan <= 2:  # gpsimd free until the first A2A
                                eng = q3[cb % 3]
                            else:
                                eng = nc.sync if cb % 2 == 0 else nc.scalar
                            if pan == 0:
                                # chunk 0 only reads cols 0:512: land all
                                # first-halves before any second half so
                                # the warm start begins ~5us earlier
                                eng.dma_start(xpan[:, cb, 0:TCH],
                                              xT_v[:, cb, 0:TCH])
                            else:
                                eng.dma_start(
                                    xpan[:, cb, :],
                                    xT_v[:, cb, pan * PAN:(pan + 1) * PAN])
                        if pan == 0:
                            for cb in range(CB):
                                q3[cb % 3].dma_start(
                                    xpan[:, cb, TCH:PAN],
                                    xT_v[:, cb, TCH:PAN])
                        if pan == 0:
                            # rest of the tables behind the critical first
                            # weight/x loads (chunk 1 needs col 512 at
                            # ~39us; sync/scalar deliver by ~31us)
                            nc.sync.dma_start(cos_sb[:, TCH:T],
                                              cosT_d[:, TCH:T])
                            nc.scalar.dma_start(sin_sb[:, TCH:T],
                                                sinS_d[:, TCH:T])

                        for half in range(PAN // TCH):
                            cc = pc * (PAN // TCH) + half
                            t0 = cc * TCH
                            h0 = half * TCH

                            def xsl(cb):
                                return xpan[:, cb, h0:h0 + TCH]

                            if pan == 0 and half == 0:
                                # cb-outer warm start: consume each cb the
                                # moment its DMA lands (8 psum groups live)
                                ps_f = [psqk.tile([128, TCH], f32, tag="qk",
                                                  name=f"psqk_0_{f}")
                                        for f in range(4)]
                                pv_t = [psv.tile([128, 2 * HD], f32, tag="v",
                                                 name=f"psv_0_{tb}")
                                        for tb in range(4)]
                                for cb in range(CB):
                                    for f in range(4):
                                        nc.tensor.matmul(
                                            ps_f[f][:],
                                            lhsT=wqk_sb[:, cb,
                                                        f * 128:(f + 1) * 128],
                                            rhs=xsl(cb), start=(cb == 0),
                                            stop=(cb == CB - 1),
                                            skip_group_check=True)
                                    for tb in range(4):
                                        nc.tensor.matmul(
                                            pv_t[tb][:],
                                            lhsT=xpan[:, cb,
                                                      h0 + tb * 128:
                                                      h0 + (tb + 1) * 128],
                                            rhs=wqk_sb[:, cb,
                                                       4 * 128:6 * 128],
                                            start=(cb == 0),
                                            stop=(cb == CB - 1),
                                            skip_group_check=True)
                                for f in range(4):
                                    rope_emit(ps_f[f], f, b, t0)
                                for tb in range(4):
                                    nc.scalar.copy(
                                        V[b][:, cc * 4 + tb, :], pv_t[tb][:])
                                continue

                            last_chunk = (pan == NPAN - 1 and half == 1)
                            for f in range(4):  # q_h0 q_h1 k_h0 k_h1
                                ps = psqk.tile([128, TCH], f32, tag="qk",
                                               name=f"psqk_{pan}_{half}_{f}")
                                for cb in range(CB):
                                    nc.tensor.matmul(
                                        ps[:],
                                        lhsT=wqk_sb[:, cb,
                                                    f * 128:(f + 1) * 128],
                                        rhs=xsl(cb),
                                        start=(cb == 0), stop=(cb == CB - 1))
                                rope_emit(ps, f, b, t0, dve_copy=last_chunk)

                            for tb in range(4):  # v
                                pv = psv.tile([128, 2 * HD], f32, tag="v",
                                              name=f"psv_{pan}_{half}_{tb}")
                                for cb in range(CB):
                                    nc.tensor.matmul(
                                        pv[:],
                                        lhsT=xpan[:, cb,
                                                  h0 + tb * 128:
                                                  h0 + (tb + 1) * 128],
                                        rhs=wqk_sb[:, cb, 4 * 128:6 * 128],
                                        start=(cb == 0), stop=(cb == CB - 1))
                                nc.scalar.copy(V[b][:, cc * 4 + tb, :],
                                               pv[:])

                # -------- Phase 2: attention + resharding -----------------
                attnall_t = []
                wout_pre = {}
                with (
                    tc.tile_pool(name="attn", bufs=2) as attnp,
                    tc.tile_pool(name="wout", bufs=2) as woutp,
                ):
                  def wfetch(part, oc):
                      w = woutp.tile([128, CB, TCH], bf16, tag="wout",
                                     name=f"wout_{part}_{oc}")
                      nc.scalar.dma_start(
                          w[:, 0:CB // 2, :],
                          woutT_v[:, 0:CB // 2, oc * TCH:(oc + 1) * TCH])
                      nc.sync.dma_start(
                          w[:, CB // 2:CB, :],
                          woutT_v[:, CB // 2:CB, oc * TCH:(oc + 1) * TCH])
                      wout_pre[(part, oc)] = w

                  # prefetch first two out-proj weight tiles on the (now
                  # idle) scalar+sync queues, ahead of the exp stream
                  for oc in range(2):
                      wfetch(0, oc)
                  with (
                    tc.tile_pool(name="ps_st", bufs=2, space="PSUM") as psst,
                    tc.tile_pool(name="ps_acc", bufs=2, space="PSUM") as psacc,
                    tc.tile_pool(name="ps_on", bufs=2, space="PSUM") as psones,
                    tc.tile_pool(name="pexp", bufs=6) as pexpp,
                    tc.tile_pool(name="pacc", bufs=2) as accp,
                    tc.tile_pool(name="onorm", bufs=3) as onp,
                  ):
                    pend_ep = [None]  # deferred epilogue of the prev chunk

                    def run_pend():
                        if pend_ep[0] is not None:
                            pend_ep[0]()
                            pend_ep[0] = None

                    for part in range(2):
                        # layout [128, i(core), hl, t] == attnallT c order
                        attnall = attnp.tile([128, CB // 2, HL, TQ], bf16,
                                             tag="attnall",
                                             name=f"attnall_{part}")
                        attnall_t.append(attnall)
                        for hl in range(HL):
                            # b-outer: alternate small/large tq chunks so
                            # DVE/ACT epilogues hide under big-chunk PE time
                            for b in range(B):
                                for tqc in range(part, CHB, 2):
                                    pend_ep[0] = _attn_chunk(
                                        nc, mybir, psst, psacc, psones,
                                        pexpp, accp, onp,
                                        qT, kT, V, mask_sb, ones_sb,
                                        a2a_in[part][hl], b, hl, tqc,
                                        SCALE, Exp, f32, bf16, run_pend)
                            # flush the group's last epilogue before its A2A
                            run_pend()
                            # reshard this (half, head) while the rest of
                            # attention / the output projection runs
                            nc.gpsimd.collective_compute(
                                "AllToAll", mybir.AluOpType.bypass,
                                replica_groups=[list(range(NCORES))],
                                ins=[a2a_in[part][hl].opt()],
                                outs=[a2a_out[part][hl].opt()],
                            )
                            # post-collective load on gpsimd: idle between triggers,
                            # and the collectives serialize on the CC
                            # engine anyway, so this delays nothing
                            nc.gpsimd.dma_start(
                                attnall[:, :, hl, :],
                                a2a_out[part][hl].rearrange(
                                    "i p t -> p i t"))

                  # -------- Phase 3: output projection --------------------
                  with (
                    tc.tile_pool(name="ps_out", bufs=2, space="PSUM") as pso,
                    tc.tile_pool(name="o3", bufs=3) as o3p,
                  ):
                    last_mm = None
                    first_mm_p1 = None
                    order = [(p, o) for p in range(2) for o in range(4)]
                    for oi, (part, oc) in enumerate(order):
                        attnall = attnall_t[part]
                        if True:
                            w = wout_pre.pop((part, oc))
                            # pipeline the NEXT weight fetch ahead of this
                            # tile's stores so its transfer starts the
                            # moment its ring slot frees (one oc period
                            # early) instead of queueing behind ~2MB of
                            # output stores -- this was a ~5us PE stall
                            if oi + 2 < len(order):
                                wfetch(*order[oi + 2])
                            for tb in range(TQ // 128):
                                po = pso.tile([128, TCH], f32, tag="out",
                                              name=f"po_{part}_{oc}_{tb}")
                                for cb in range(CB):
                                    mm = nc.tensor.matmul(
                                        po[:],
                                        lhsT=attnall[:, cb // 2, cb % 2,
                                                     tb * 128:(tb + 1) * 128],
                                        rhs=w[:, cb, :],
                                        start=(cb == 0),
                                        stop=(cb == CB - 1))
                                    if part == 1 and first_mm_p1 is None:
                                        first_mm_p1 = mm
                                    if part == 0:
                                        last_mm = mm
                                ot = o3p.tile([128, TCH], f32, tag="o3",
                                              name=f"ot_{part}_{oc}_{tb}")
                                # DVE copy: ACT stays free for exp tails
                                nc.vector.tensor_scalar_add(ot[:], po[:],
                                                            0.0)
                                # alternate store queues so neither queue
                                # carries more than ~1MB per oc period and
                                # the pipelined wout fetches stay ahead
                                st_eng = nc.scalar if tb % 2 == 0 else nc.sync
                                st_eng.dma_start(
                                    out_d[part * TQ + tb * 128:
                                          part * TQ + (tb + 1) * 128,
                                          oc * TCH:(oc + 1) * TCH],
                                    ot[:])
                    # keep the two out-proj halves in emission order on the
                    # PE so part 1 (gated on the later collectives) cannot
                    # starve part 0's remaining matmuls
                    tile.add_dep_helper(
                        first_mm_p1.ins, last_mm.ins, sync=False,
                        reason="outproj part order")

    nc.compile()
    return nc


def _attn_chunk(nc, mybir, psst, psacc, psones, pexpp, accp, onp, qT, kT, V,
                mask_sb, ones_sb, a2a_in_ph, b, hl, tqc, SCALE, Exp, f32,
                bf16, run_pend):
    """One (batch, head, 512-query-chunk) of causal attention.

    Emits scores/exp/AV/denominator work and returns an epilogue closure
    (fold + ones-matmul + recip + oT + store).  The caller runs the
    epilogue only after the NEXT chunk's score pairs are on the PE queue,
    so the PE never stalls waiting for the DVE to finish the denominator
    (the previous ~0.6us bubble at every chunk boundary)."""
    ntk = (tqc + 1) * (TCH // 128)
    npair = ntk // 2
    q_sl = qT[hl][b][:, tqc * TCH:(tqc + 1) * TCH]
    av = psacc.tile([128, TCH], f32, tag="acc", name=f"av_{b}_{hl}_{tqc}")
    # denominator partials, one 512-wide lane-sum strip per pair half
    acc2 = accp.tile([128, 2, TCH], bf16, tag="pacc",
                     name=f"pa_{b}_{hl}_{tqc}")
    accf = accp.tile([128, TCH], bf16, tag="paccf",
                     name=f"pf_{b}_{hl}_{tqc}")
    pexp_t = {}
    acc_init = [True, True]

    def col0(j):
        """First valid tq column for tk-block j (causal: tq >= tk)."""
        jd = j - (TCH // 128) * tqc
        return 128 * jd if jd > 0 else 0

    def emit_pair(p):
        """scores -> exp -> mask for two 128-token tk blocks."""
        st = psst.tile([128, 2, TCH], f32, tag="st",
                       name=f"st_{b}_{hl}_{tqc}_{p}")
        pe = pexpp.tile([128, 2, TCH], bf16, tag="pexp",
                        name=f"pe_{b}_{hl}_{tqc}_{p}")
        for jj in range(2):
            j = 2 * p + jj
            c0 = col0(j)
            nc.tensor.matmul(
                st[:, jj, c0:TCH],
                lhsT=kT[hl][b][:, j * 128:(j + 1) * 128],
                rhs=q_sl[:, c0:TCH], start=True, stop=True)
        if col0(2 * p) == 0 and col0(2 * p + 1) == 0:
            nc.scalar.activation(pe[:], st[:], Exp, scale=SCALE)
        else:
            for jj in range(2):
                c0 = col0(2 * p + jj)
                nc.scalar.activation(
                    pe[:, jj, c0:TCH], st[:, jj, c0:TCH], Exp, scale=SCALE)
        for jj in range(2):
            j = 2 * p + jj
            jd = j - (TCH // 128) * tqc
            if jd >= 0:
                # causal mask: only the 128-col diagonal band is partial
                c0 = col0(j)
                nc.vector.tensor_mul(
                    pe[:, jj, c0:c0 + 128], pe[:, jj, c0:c0 + 128],
                    mask_sb[:, jd, c0:c0 + 128])
        pexp_t[p] = pe

    PIPEP = 2  # score/exp pairs emitted ahead of their AV consumption
    for p in range(min(PIPEP, npair)):
        emit_pair(p)
    # previous chunk's epilogue rides behind BOTH our prefill score pairs
    # (~1.1us of PE work): by then its DVE denominator folds have drained,
    # so its ones-matmul and oT production cost the PE nothing
    run_pend()
    for p in range(npair):
        if p + PIPEP < npair:
            emit_pair(p + PIPEP)
        pe = pexp_t.pop(p)
        for jj in range(2):
            j = 2 * p + jj
            c0 = col0(j)
            first = j == 0  # always full width: sets has_written everywhere
            last = j == ntk - 1
            nc.tensor.matmul(
                av[:, c0:TCH],
                lhsT=V[b][:, j, hl * 128:(hl + 1) * 128],
                rhs=pe[:, jj, c0:TCH],
                start=first, stop=last, skip_group_check=True)
        # softmax-denominator partials on the DVE (per tk-lane, per half)
        if col0(2 * p) == 0 and col0(2 * p + 1) == 0:
            if acc_init[0]:
                nc.vector.tensor_scalar_add(acc2[:], pe[:], 0.0)
                acc_init[0] = acc_init[1] = False
            else:
                nc.vector.tensor_add(acc2[:], acc2[:], pe[:])
        else:
            for jj in range(2):
                c0 = col0(2 * p + jj)
                dst = acc2[:, jj, c0:TCH]
                src = pe[:, jj, c0:TCH]
                if acc_init[jj]:
                    nc.vector.tensor_scalar_add(dst, src, 0.0)
                    acc_init[jj] = False
                else:
                    nc.vector.tensor_add(dst, dst, src)

    # fold halves on the DVE now (they chase the AV matmuls), but defer the
    # PE/DVE tail into an epilogue the caller schedules after the next
    # chunk's first score pairs
    if tqc == 0:  # acc2[:,1,0:128] never written (tk block 1 starts at 128)
        nc.vector.tensor_scalar_add(accf[:, 0:128], acc2[:, 0, 0:128], 0.0)
        nc.vector.tensor_add(accf[:, 128:], acc2[:, 0, 128:],
                             acc2[:, 1, 128:])
    else:
        nc.vector.tensor_add(accf[:], acc2[:, 0, :], acc2[:, 1, :])

    def epilogue():
        ones_ps = psones.tile([128, TCH], f32, tag="ones",
                              name=f"on_{b}_{hl}_{tqc}")
        nc.tensor.matmul(ones_ps[:], lhsT=ones_sb[:], rhs=accf[:],
                         start=True, stop=True)
        recip = onp.tile([128, TCH], f32, tag="recip",
                         name=f"rc_{b}_{hl}_{tqc}")
        nc.vector.reciprocal_approx_fast(recip[:], ones_ps[:])
        oT = onp.tile([128, TCH], bf16, tag="oT", name=f"oT_{b}_{hl}_{tqc}")
        nc.vector.tensor_mul(oT[:], av[:], recip[:])
        dj = b * 2 + tqc // 2
        nc.sync.dma_start(a2a_in_ph[dj, :, :], oT[:])

    return epilogue


def prep_inputs(x, cos, sin, w_qkv, w_out, T=2048):
    """Host-side shard/layout prep. Returns in_maps for the 8 cores."""
    TOK = B * T
    xT = np.ascontiguousarray(x.reshape(TOK, D).T).astype(BF16)
    cosT = np.ascontiguousarray(cos.T[_PERM, :]).astype(BF16)
    sinS = np.ascontiguousarray(sin.T[_PERM, :] * _SIGN[:, None]).astype(BF16)
    woutT = np.ascontiguousarray(w_out.T).astype(BF16)
    in_maps = []
    for c in range(NCORES):
        rows = []
        for sec in range(2):  # q, k (perm'd)
            for hl in range(HL):
                h = 2 * c + hl
                w = w_qkv[sec * D + h * HD:sec * D + (h + 1) * HD, :]
                rows.append(w[_PERM, :])
        for hl in range(HL):  # v natural
            h = 2 * c + hl
            rows.append(w_qkv[2 * D + h * HD:2 * D + (h + 1) * HD, :])
        wqkT = np.ascontiguousarray(np.concatenate(rows, 0).T).astype(BF16)
        in_maps.append({"xT": xT, "wqkT": wqkT, "woutT": woutT,
                        "cosT": cosT, "sinS": sinS})
    return in_maps


_NC_CACHE = {}


def _get_nc(T=2048):
    if T not in _NC_CACHE:
        _NC_CACHE[T] = build_nc(T)
    return _NC_CACHE[T]


def kernel(x, cos, sin, w_qkv, w_out):
    import concourse.bass_utils as bass_utils

    T = x.shape[1]
    x = np.asarray(x, np.float32)
    cos = np.asarray(cos, np.float32)
    sin = np.asarray(sin, np.float32)
    w_qkv = np.asarray(w_qkv, np.float32)
    w_out = np.asarray(w_out, np.float32)

    nc = _get_nc(T)
    in_maps = prep_inputs(x, cos, sin, w_qkv, w_out, T)
    res = bass_utils.run_bass_kernel_spmd(nc, in_maps,
                                          core_ids=list(range(NCORES)))
    THALF = T // 2
    full = np.empty((B, T, D), np.float32)
    for j in range(NCORES):
        b, hf = divmod(j, 2)
        full[b, hf * THALF:(hf + 1) * THALF, :] = res.results[j]["out"]
    return full



# revision 25
# speedup vs baseline: 1.3597x; 1.0020x over previous
"""Causal self-attention (B=4, T=2048, D=2048, H=16, HD=128) on 8 Trainium2
NeuronCores.

Sharding: Megatron-style tensor parallel over heads for QKV projection +
attention (2 heads per core), then on-device AllToAlls reshard from
head-parallel to token-parallel (core j owns tokens of batch j//2, half j%2)
for the output projection.  Host only slices/transposes weights, replicates
activations, and concatenates the 8 output shards.

Device layouts (all matmul operands bf16, fp32 PSUM accumulation):
  xT    [D, B*T]   x transposed (contraction dim on partitions)
  qT/kT [128, T]   per (local head, batch); d-order permuted so the RoPE
                   rotate-half partner sits 16 partitions away (within a
                   32-partition quadrant, reachable by DVE stream_shuffle).
                   Any consistent permutation of d leaves q.k unchanged.
  V     [T, 128]   natural d order (feeds AV matmul lhsT and out-proj order)
  S^T   [tk, tq]   scores transposed; no max-subtraction needed (logits ~
                   N(0,1), bounded ~ +-6, exp can't overflow).

The softmax denominator is accumulated on the DVE (acc[tk-lane, tq] +=
exp-tile per tk-block) and reduced across partitions by a single
128-column-stationary ones-matmul per 512-query chunk, instead of a
full-width ones-matmul per tk-block on the PE (saves ~9% PE columns; the
PE runs power-throttled at ~1.94 GHz so PE columns are the wall clock).

The attention loop runs tq-half 0 (even 512-token chunks) then half 1, with
one AllToAll per (half, head) issued as soon as that head's chunks finish —
all four collectives overlap the remaining attention / output projection.
The gpsimd queue carries ONLY the collective triggers; weight/activation
DMAs ride the sync + scalar HWDGE queues.
"""

import sys

for _p in ("/opt/trn_rl_repo", "/root/.axon_site/_ro/trn_rl_repo"):
    if _p not in sys.path:
        sys.path.insert(0, _p)

import numpy as np
import ml_dtypes

BF16 = ml_dtypes.bfloat16

B = 4
D = 2048
H = 16
HD = 128
NCORES = 8
HL = 2           # heads per core
CB = D // 128    # contraction blocks
TCH = 512        # token chunk (matmul moving free dim)
PAN = 1024       # x DMA panel (2 chunks; 2KB/partition DMA lines)


def _perm128():
    """Partition order for q/k head dims: quadrant g holds dims
    [16g,16g+16) (lo) then [64+16g, 64+16g+16) (hi), so the rotate-half
    partner of partition p is p+-16 (same 32-partition quadrant)."""
    perm = np.zeros(128, np.int64)
    for p in range(128):
        g, i = divmod(p, 32)
        perm[p] = g * 16 + i if i < 16 else 64 + g * 16 + (i - 16)
    return perm


_PERM = _perm128()
_SHUF = [(i + 16) % 32 for i in range(32)]  # out[i] = in[(i+16)%32]
_SIGN = np.where(np.arange(128) % 32 < 16, -1.0, 1.0).astype(np.float32)


def build_nc(T=2048):
    import concourse.bacc as bacc
    import concourse.tile as tile
    import concourse.mybir as mybir

    f32 = mybir.dt.float32
    bf16 = mybir.dt.bfloat16
    TOK = B * T
    THALF = T // 2
    TQ = THALF // 2           # tokens per (core, a2a part)
    NPAN = TOK // PAN         # x panels total
    CHB = T // TCH            # token chunks per batch
    TB = T // 128             # 128-token blocks per batch
    SCALE = float(HD) ** -0.5
    Exp = mybir.ActivationFunctionType.Exp

    assert TQ == TCH, "A2A split layout assumes T == 2048"
    nc = bacc.Bacc("TRN2", target_bir_lowering=False, debug=False,
                   num_devices=NCORES)

    xT_d = nc.dram_tensor("xT", [D, TOK], bf16, kind="ExternalInput")
    wqkT_d = nc.dram_tensor("wqkT", [D, 6 * HD], bf16, kind="ExternalInput")
    woutT_d = nc.dram_tensor("woutT", [D, D], bf16, kind="ExternalInput")
    cosT_d = nc.dram_tensor("cosT", [HD, T], bf16, kind="ExternalInput")
    sinS_d = nc.dram_tensor("sinS", [HD, T], bf16, kind="ExternalInput")
    out_d = nc.dram_tensor("out", [THALF, D], f32, kind="ExternalOutput")

    xT_v = xT_d.ap().rearrange("(cb p) t -> p cb t", p=128)
    wqkT_v = wqkT_d.ap().rearrange("(cb p) f -> p cb f", p=128)
    woutT_v = woutT_d.ap().rearrange("(cb p) o -> p cb o", p=128)

    with tile.TileContext(nc) as tc:
        with (
            tc.tile_pool(name="const", bufs=1) as constp,
            tc.tile_pool(name="dram", bufs=1, space="DRAM") as dramp,
        ):
            cos_sb = constp.tile([128, T], bf16, name="cos_sb")
            sin_sb = constp.tile([128, T], bf16, name="sin_sb")
            mask_sb = constp.tile([128, 4, TCH], bf16, name="mask_sb")
            ones_sb = constp.tile([128, 128], bf16, name="ones_sb")
            nc.gpsimd.memset(mask_sb[:], 1.0)
            for jd in range(4):
                # keep 1.0 where  tq_rel - tk_rel - 128*jd >= 0  else 0
                nc.gpsimd.affine_select(
                    out=mask_sb[:, jd, :], in_=mask_sb[:, jd, :],
                    compare_op=mybir.AluOpType.is_ge, fill=0.0,
                    base=-128 * jd, pattern=[[1, TCH]], channel_multiplier=-1,
                )
            nc.gpsimd.memset(ones_sb[:], 1.0)

            # per (tq-half, local head) AllToAll bounce buffers
            a2a_in = [[dramp.tile([NCORES, 128, TQ], bf16,
                                  name=f"a2a_in{p}{h}") for h in range(HL)]
                      for p in range(2)]
            a2a_out = [[dramp.tile([NCORES, 128, TQ], bf16,
                                   name=f"a2a_out{p}{h}") for h in range(HL)]
                       for p in range(2)]

            with tc.tile_pool(name="qkv", bufs=1) as qkvp:
                qT = [[qkvp.tile([128, T], bf16, name=f"qT_{hl}_{b}")
                       for b in range(B)] for hl in range(HL)]
                kT = [[qkvp.tile([128, T], bf16, name=f"kT_{hl}_{b}")
                       for b in range(B)] for hl in range(HL)]
                V = [qkvp.tile([128, TB, 2 * HD], bf16, name=f"V_{b}")
                     for b in range(B)]

                # -------- Phase 1: QKV projection + RoPE ------------------
                with (
                    tc.tile_pool(name="wqk", bufs=1) as wqkp,
                    tc.tile_pool(name="xin", bufs=2) as xp,
                    tc.tile_pool(name="ps_qk", bufs=4, space="PSUM") as psqk,
                    tc.tile_pool(name="ps_v", bufs=4, space="PSUM") as psv,
                    tc.tile_pool(name="rope", bufs=3) as ropep,
                ):
                    wqk_sb = wqkp.tile([128, CB, 6 * HD], bf16,
                                       name="wqk_sb")
                    # startup: round-robin (wqk cb, x cb) bundles over all
                    # three DMA queues so MM(cb) can start the moment its
                    # weight+activation blocks land (gpsimd queue is free
                    # until the first collective, ~450us in)
                    q3 = [nc.sync, nc.scalar, nc.gpsimd]

                    def rope_emit(ps, f, b, t0, dve_copy=False):
                        qraw = ropep.tile([128, TCH], bf16, tag="qraw",
                                          name=f"qraw_{b}_{t0}_{f}")
                        if dve_copy:
                            # last chunk: keep ACT clear so the first
                            # attention exps aren't stuck behind its copies
                            nc.vector.tensor_scalar_add(qraw[:], ps[:], 0.0)
                        else:
                            nc.scalar.copy(qraw[:], ps[:])
                        rot = ropep.tile([128, TCH], bf16, tag="rot",
                                         name=f"rot_{b}_{t0}_{f}")
                        nc.vector.stream_shuffle(rot[:], qraw[:],
                                                 mask=_SHUF)
                        t1 = ropep.tile([128, TCH], bf16, tag="t1",
                                        name=f"t1_{b}_{t0}_{f}")
                        nc.vector.tensor_mul(
                            t1[:], qraw[:], cos_sb[:, t0:t0 + TCH])
                        nc.vector.tensor_mul(
                            rot[:], rot[:], sin_sb[:, t0:t0 + TCH])
                        dest = (qT if f < 2 else kT)[f % 2][b]
                        nc.vector.tensor_add(
                            dest[:, t0:t0 + TCH], t1[:], rot[:])

                    # rope tables: chunk 0's columns ride at the head of the
                    # (otherwise idle) gpsimd queue -- 256KB, small enough
                    # not to delay the warm-start cb loads behind them --
                    # so chunk 0's rope_emit never waits on cos/sin.
                    nc.gpsimd.dma_start(cos_sb[:, 0:TCH], cosT_d[:, 0:TCH])
                    nc.gpsimd.dma_start(sin_sb[:, 0:TCH], sinS_d[:, 0:TCH])

                    for pan in range(NPAN):
                        b, pc = divmod(pan, T // PAN)
                        xpan = xp.tile([128, CB, PAN], bf16, tag="xpan",
                                       name=f"xpan_{pan}")
                        for cb in range(CB):
                            if pan == 0:
                                q3[cb % 3].dma_start(wqk_sb[:, cb, :],
                                                     wqkT_v[:, cb, :])
                            if pan <= 2:  # gpsimd free until the first A2A
                                eng = q3[cb % 3]
                            else:
                                eng = nc.sync if cb % 2 == 0 else nc.scalar
                            eng.dma_start(
                                xpan[:, cb, :],
                                xT_v[:, cb, pan * PAN:(pan + 1) * PAN])
                        if pan == 0:
                            # rest of the rope tables behind pan 0 on the
                            # gpsimd queue (chunk 1 needs col 512 at ~39us,
                            # pan 1 chunks need cols 1024+ at ~64us)
                            nc.gpsimd.dma_start(cos_sb[:, TCH:PAN],
                                                cosT_d[:, TCH:PAN])
                            nc.gpsimd.dma_start(sin_sb[:, TCH:PAN],
                                                sinS_d[:, TCH:PAN])
                            nc.gpsimd.dma_start(cos_sb[:, PAN:T],
                                                cosT_d[:, PAN:T])
                            nc.gpsimd.dma_start(sin_sb[:, PAN:T],
                                                sinS_d[:, PAN:T])

                        for half in range(PAN // TCH):
                            cc = pc * (PAN // TCH) + half
                            t0 = cc * TCH
                            h0 = half * TCH

                            def xsl(cb):
                                return xpan[:, cb, h0:h0 + TCH]

                            if pan == 0 and half == 0:
                                # cb-outer warm start: consume each cb the
                                # moment its DMA lands (8 psum groups live)
                                ps_f = [psqk.tile([128, TCH], f32, tag="qk",
                                                  name=f"psqk_0_{f}")
                                        for f in range(4)]
                                pv_t = [psv.tile([128, 2 * HD], f32, tag="v",
                                                 name=f"psv_0_{tb}")
                                        for tb in range(4)]
                                for cb in range(CB):
                                    for f in range(4):
                                        nc.tensor.matmul(
                                            ps_f[f][:],
                                            lhsT=wqk_sb[:, cb,
                                                        f * 128:(f + 1) * 128],
                                            rhs=xsl(cb), start=(cb == 0),
                                            stop=(cb == CB - 1),
                                            skip_group_check=True)
                                    for tb in range(4):
                                        nc.tensor.matmul(
                                            pv_t[tb][:],
                                            lhsT=xpan[:, cb,
                                                      h0 + tb * 128:
                                                      h0 + (tb + 1) * 128],
                                            rhs=wqk_sb[:, cb,
                                                       4 * 128:6 * 128],
                                            start=(cb == 0),
                                            stop=(cb == CB - 1),
                                            skip_group_check=True)
                                for f in range(4):
                                    rope_emit(ps_f[f], f, b, t0)
                                for tb in range(4):
                                    nc.scalar.copy(
                                        V[b][:, cc * 4 + tb, :], pv_t[tb][:])
                                continue

                            last_chunk = (pan == NPAN - 1 and half == 1)
                            for f in range(4):  # q_h0 q_h1 k_h0 k_h1
                                ps = psqk.tile([128, TCH], f32, tag="qk",
                                               name=f"psqk_{pan}_{half}_{f}")
                                for cb in range(CB):
                                    nc.tensor.matmul(
                                        ps[:],
                                        lhsT=wqk_sb[:, cb,
                                                    f * 128:(f + 1) * 128],
                                        rhs=xsl(cb),
                                        start=(cb == 0), stop=(cb == CB - 1))
                                rope_emit(ps, f, b, t0, dve_copy=last_chunk)

                            for tb in range(4):  # v
                                pv = psv.tile([128, 2 * HD], f32, tag="v",
                                              name=f"psv_{pan}_{half}_{tb}")
                                for cb in range(CB):
                                    nc.tensor.matmul(
                                        pv[:],
                                        lhsT=xpan[:, cb,
                                                  h0 + tb * 128:
                                                  h0 + (tb + 1) * 128],
                                        rhs=wqk_sb[:, cb, 4 * 128:6 * 128],
                                        start=(cb == 0), stop=(cb == CB - 1))
                                if last_chunk:
                                    # keep ACT clear for the first exps
                                    nc.vector.tensor_scalar_add(
                                        V[b][:, cc * 4 + tb, :], pv[:], 0.0)
                                else:
                                    nc.scalar.copy(V[b][:, cc * 4 + tb, :],
                                                   pv[:])

                # -------- Phase 2: attention + resharding -----------------
                attnall_t = []
                wout_pre = {}
                with (
                    tc.tile_pool(name="attn", bufs=2) as attnp,
                    tc.tile_pool(name="wout", bufs=2) as woutp,
                ):
                  def wfetch(part, oc, sync_only=False):
                      # sync_only for the two pre-attention fetches: their
                      # transfers have ~180us of slack, and a trigger on the
                      # scalar queue would block the first exps ~0.7us each
                      w = woutp.tile([128, CB, TCH], bf16, tag="wout",
                                     name=f"wout_{part}_{oc}")
                      eng0 = nc.sync if sync_only else nc.scalar
                      eng0.dma_start(
                          w[:, 0:CB // 2, :],
                          woutT_v[:, 0:CB // 2, oc * TCH:(oc + 1) * TCH])
                      nc.sync.dma_start(
                          w[:, CB // 2:CB, :],
                          woutT_v[:, CB // 2:CB, oc * TCH:(oc + 1) * TCH])
                      wout_pre[(part, oc)] = w

                  # prefetch first two out-proj weight tiles ahead of the
                  # exp stream
                  for oc in range(2):
                      wfetch(0, oc, sync_only=True)
                  with (
                    tc.tile_pool(name="ps_st", bufs=2, space="PSUM") as psst,
                    tc.tile_pool(name="ps_acc", bufs=2, space="PSUM") as psacc,
                    tc.tile_pool(name="ps_on", bufs=2, space="PSUM") as psones,
                    tc.tile_pool(name="pexp", bufs=6) as pexpp,
                    tc.tile_pool(name="pacc", bufs=2) as accp,
                    tc.tile_pool(name="onorm", bufs=3) as onp,
                  ):
                    pend_ep = [None]  # deferred epilogue of the prev chunk

                    def run_pend():
                        if pend_ep[0] is not None:
                            pend_ep[0]()
                            pend_ep[0] = None

                    for part in range(2):
                        # layout [128, i(core), hl, t] == attnallT c order
                        attnall = attnp.tile([128, CB // 2, HL, TQ], bf16,
                                             tag="attnall",
                                             name=f"attnall_{part}")
                        attnall_t.append(attnall)
                        for hl in range(HL):
                            # b-outer: alternate small/large tq chunks so
                            # DVE/ACT epilogues hide under big-chunk PE time
                            for b in range(B):
                                for tqc in range(part, CHB, 2):
                                    pend_ep[0] = _attn_chunk(
                                        nc, mybir, psst, psacc, psones,
                                        pexpp, accp, onp,
                                        qT, kT, V, mask_sb, ones_sb,
                                        a2a_in[part][hl], b, hl, tqc,
                                        SCALE, Exp, f32, bf16, run_pend)
                            # flush the group's last epilogue before its A2A
                            run_pend()
                            # reshard this (half, head) while the rest of
                            # attention / the output projection runs
                            nc.gpsimd.collective_compute(
                                "AllToAll", mybir.AluOpType.bypass,
                                replica_groups=[list(range(NCORES))],
                                ins=[a2a_in[part][hl].opt()],
                                outs=[a2a_out[part][hl].opt()],
                            )
                            # post-collective load on gpsimd: idle between
                            # triggers, and the collectives serialize on the
                            # CC engine anyway.  Split so the out-proj's
                            # first contraction blocks (low i) land first.
                            for ih in range(2):
                                nc.gpsimd.dma_start(
                                    attnall[:, 4 * ih:4 * ih + 4, hl, :],
                                    a2a_out[part][hl][4 * ih:4 * ih + 4]
                                    .rearrange("i p t -> p i t"))

                  # -------- Phase 3: output projection --------------------
                  with (
                    tc.tile_pool(name="ps_out", bufs=2, space="PSUM") as pso,
                    tc.tile_pool(name="o3", bufs=3) as o3p,
                  ):
                    last_mm = None
                    first_mm_p1 = None
                    order = [(p, o) for p in range(2) for o in range(4)]
                    for oi, (part, oc) in enumerate(order):
                        attnall = attnall_t[part]
                        if True:
                            w = wout_pre.pop((part, oc))
                            # pipeline the NEXT weight fetch ahead of this
                            # tile's stores so its transfer starts the
                            # moment its ring slot frees (one oc period
                            # early) instead of queueing behind ~2MB of
                            # output stores -- this was a ~5us PE stall
                            if oi + 2 < len(order):
                                wfetch(*order[oi + 2])
                            for tb in range(TQ // 128):
                                po = pso.tile([128, TCH], f32, tag="out",
                                              name=f"po_{part}_{oc}_{tb}")
                                for cb in range(CB):
                                    mm = nc.tensor.matmul(
                                        po[:],
                                        lhsT=attnall[:, cb // 2, cb % 2,
                                                     tb * 128:(tb + 1) * 128],
                                        rhs=w[:, cb, :],
                                        start=(cb == 0),
                                        stop=(cb == CB - 1))
                                    if part == 1 and first_mm_p1 is None:
                                        first_mm_p1 = mm
                                    if part == 0:
                                        last_mm = mm
                                ot = o3p.tile([128, TCH], f32, tag="o3",
                                              name=f"ot_{part}_{oc}_{tb}")
                                # DVE copy: ACT stays free for exp tails
                                nc.vector.tensor_scalar_add(ot[:], po[:],
                                                            0.0)
                                # alternate store queues so neither queue
                                # carries more than ~1MB per oc period and
                                # the pipelined wout fetches stay ahead
                                st_eng = nc.scalar if tb % 2 == 0 else nc.sync
                                st_eng.dma_start(
                                    out_d[part * TQ + tb * 128:
                                          part * TQ + (tb + 1) * 128,
                                          oc * TCH:(oc + 1) * TCH],
                                    ot[:])
                    # keep the two out-proj halves in emission order on the
                    # PE so part 1 (gated on the later collectives) cannot
                    # starve part 0's remaining matmuls
                    tile.add_dep_helper(
                        first_mm_p1.ins, last_mm.ins, sync=False,
                        reason="outproj part order")

    nc.compile()
    return nc


def _attn_chunk(nc, mybir, psst, psacc, psones, pexpp, accp, onp, qT, kT, V,
                mask_sb, ones_sb, a2a_in_ph, b, hl, tqc, SCALE, Exp, f32,
                bf16, run_pend):
    """One (batch, head, 512-query-chunk) of causal attention.

    Emits scores/exp/AV/denominator work and returns an epilogue closure
    (fold + ones-matmul + recip + oT + store).  The caller runs the
    epilogue only after the NEXT chunk's score pairs are on the PE queue,
    so the PE never stalls waiting for the DVE to finish the denominator
    (the previous ~0.6us bubble at every chunk boundary)."""
    ntk = (tqc + 1) * (TCH // 128)
    npair = ntk // 2
    q_sl = qT[hl][b][:, tqc * TCH:(tqc + 1) * TCH]
    av = psacc.tile([128, TCH], f32, tag="acc", name=f"av_{b}_{hl}_{tqc}")
    # denominator partials: ONE lane-sum strip, fed per tk-block.  A single
    # strip needs no final fold, so the DVE tail after the last AV matmul
    # is one 0.37us add and the deferred ones-matmul never stalls the PE.
    acc1 = accp.tile([128, TCH], bf16, tag="pacc",
                     name=f"pa_{b}_{hl}_{tqc}")
    pexp_t = {}

    def col0(j):
        """First valid tq column for tk-block j (causal: tq >= tk)."""
        jd = j - (TCH // 128) * tqc
        return 128 * jd if jd > 0 else 0

    def emit_pair(p):
        """scores -> exp -> mask for two 128-token tk blocks."""
        st = psst.tile([128, 2, TCH], f32, tag="st",
                       name=f"st_{b}_{hl}_{tqc}_{p}")
        pe = pexpp.tile([128, 2, TCH], bf16, tag="pexp",
                        name=f"pe_{b}_{hl}_{tqc}_{p}")
        for jj in range(2):
            j = 2 * p + jj
            c0 = col0(j)
            nc.tensor.matmul(
                st[:, jj, c0:TCH],
                lhsT=kT[hl][b][:, j * 128:(j + 1) * 128],
                rhs=q_sl[:, c0:TCH], start=True, stop=True)
        if col0(2 * p) == 0 and col0(2 * p + 1) == 0:
            nc.scalar.activation(pe[:], st[:], Exp, scale=SCALE)
        else:
            for jj in range(2):
                c0 = col0(2 * p + jj)
                nc.scalar.activation(
                    pe[:, jj, c0:TCH], st[:, jj, c0:TCH], Exp, scale=SCALE)
        for jj in range(2):
            j = 2 * p + jj
            jd = j - (TCH // 128) * tqc
            if jd >= 0:
                # causal mask: only the 128-col diagonal band is partial
                c0 = col0(j)
                nc.vector.tensor_mul(
                    pe[:, jj, c0:c0 + 128], pe[:, jj, c0:c0 + 128],
                    mask_sb[:, jd, c0:c0 + 128])
        pexp_t[p] = pe

    PIPEP = 2  # score/exp pairs emitted ahead of their AV consumption
    for p in range(min(PIPEP, npair)):
        emit_pair(p)
    # previous chunk's epilogue rides behind BOTH our prefill score pairs
    # (~1.1us of PE work): by then its DVE denominator folds have drained,
    # so its ones-matmul and oT production cost the PE nothing
    run_pend()
    for p in range(npair):
        if p + PIPEP < npair:
            emit_pair(p + PIPEP)
        pe = pexp_t.pop(p)
        for jj in range(2):
            j = 2 * p + jj
            c0 = col0(j)
            first = j == 0  # always full width: sets has_written everywhere
            last = j == ntk - 1
            nc.tensor.matmul(
                av[:, c0:TCH],
                lhsT=V[b][:, j, hl * 128:(hl + 1) * 128],
                rhs=pe[:, jj, c0:TCH],
                start=first, stop=last, skip_group_check=True)
        # softmax-denominator partials on the DVE (per tk-lane).  tk block
        # j=0 always spans the full chunk width, so it initializes the
        # strip and every later block accumulates into its causal range.
        for jj in range(2):
            j = 2 * p + jj
            c0 = col0(j)
            src = pe[:, jj, c0:TCH]
            if j == 0:
                nc.vector.tensor_scalar_add(acc1[:], src, 0.0)
            else:
                nc.vector.tensor_add(acc1[:, c0:TCH], acc1[:, c0:TCH], src)

    def epilogue():
        ones_ps = psones.tile([128, TCH], f32, tag="ones",
                              name=f"on_{b}_{hl}_{tqc}")
        nc.tensor.matmul(ones_ps[:], lhsT=ones_sb[:], rhs=acc1[:],
                         start=True, stop=True)
        recip = onp.tile([128, TCH], f32, tag="recip",
                         name=f"rc_{b}_{hl}_{tqc}")
        nc.vector.reciprocal_approx_fast(recip[:], ones_ps[:])
        oT = onp.tile([128, TCH], bf16, tag="oT", name=f"oT_{b}_{hl}_{tqc}")
        nc.vector.tensor_mul(oT[:], av[:], recip[:])
        dj = b * 2 + tqc // 2
        nc.sync.dma_start(a2a_in_ph[dj, :, :], oT[:])

    return epilogue


def prep_inputs(x, cos, sin, w_qkv, w_out, T=2048):
    """Host-side shard/layout prep. Returns in_maps for the 8 cores."""
    TOK = B * T
    xT = np.ascontiguousarray(x.reshape(TOK, D).T).astype(BF16)
    cosT = np.ascontiguousarray(cos.T[_PERM, :]).astype(BF16)
    sinS = np.ascontiguousarray(sin.T[_PERM, :] * _SIGN[:, None]).astype(BF16)
    woutT = np.ascontiguousarray(w_out.T).astype(BF16)
    in_maps = []
    for c in range(NCORES):
        rows = []
        for sec in range(2):  # q, k (perm'd)
            for hl in range(HL):
                h = 2 * c + hl
                w = w_qkv[sec * D + h * HD:sec * D + (h + 1) * HD, :]
                rows.append(w[_PERM, :])
        for hl in range(HL):  # v natural
            h = 2 * c + hl
            rows.append(w_qkv[2 * D + h * HD:2 * D + (h + 1) * HD, :])
        wqkT = np.ascontiguousarray(np.concatenate(rows, 0).T).astype(BF16)
        in_maps.append({"xT": xT, "wqkT": wqkT, "woutT": woutT,
                        "cosT": cosT, "sinS": sinS})
    return in_maps


_NC_CACHE = {}


def _get_nc(T=2048):
    if T not in _NC_CACHE:
        _NC_CACHE[T] = build_nc(T)
    return _NC_CACHE[T]


def kernel(x, cos, sin, w_qkv, w_out):
    import concourse.bass_utils as bass_utils

    T = x.shape[1]
    x = np.asarray(x, np.float32)
    cos = np.asarray(cos, np.float32)
    sin = np.asarray(sin, np.float32)
    w_qkv = np.asarray(w_qkv, np.float32)
    w_out = np.asarray(w_out, np.float32)

    nc = _get_nc(T)
    in_maps = prep_inputs(x, cos, sin, w_qkv, w_out, T)
    res = bass_utils.run_bass_kernel_spmd(nc, in_maps,
                                          core_ids=list(range(NCORES)))
    THALF = T // 2
    full = np.empty((B, T, D), np.float32)
    for j in range(NCORES):
        b, hf = divmod(j, 2)
        full[b, hf * THALF:(hf + 1) * THALF, :] = res.results[j]["out"]
    return full

